# revision 36
# baseline (speedup 1.0000x reference)
"""Trainium2 Bass kernel for an AttentionBlock (LN -> QKV -> attn -> out-proj + residual).

Shapes (hardcoded per problem spec): B=8, L=1024, C=1024, H=8 heads.
The reference uses a raw row-major reshape (torch-style .view) of q/k/v from
[B, L, C] to [B*H, L, C/H]; with L=1024, C=1024, H=8 this makes each
"attention head" operate on a contiguous 128-sequence-row block of the
[L, C] matrix, reinterpreted as [1024, 128].

Sharding: pure data-parallel over batch, one batch element per NeuronCore
(8 cores). No collectives.
"""

import math
from contextlib import ExitStack

import ml_dtypes
import numpy as np

import concourse.bass as bass
import concourse.bacc as bacc
import concourse.tile as tile
from concourse import mybir
from concourse import bass_utils
from concourse.masks import make_identity

L = 1024
C = 1024
H = 8          # heads; also number of 128-row l-tiles (head h <-> l-tile h)
CH = 128       # head dim
NT = 8         # l tiles (128 rows each)
NG = 8         # c groups (128 cols each)
EPS = 1e-5
S2 = 1.0 / math.sqrt(CH)   # combined q&k scale: (ch^-0.25)^2

f32 = mybir.dt.float32
f32r = mybir.dt.float32r
bf16 = mybir.dt.bfloat16
AF = mybir.ActivationFunctionType
ALU = mybir.AluOpType



def _act_reciprocal(nc, out, in_):
    """Reciprocal on ScalarE (LUT). bass blocks this func due to accuracy
    concerns; for softmax denominators the ~1e-3 LUT error is negligible and
    the single-lane DVE reciprocal (6.5us for [1,1024]) is on the critical
    path."""
    eng = nc.scalar
    ins = [eng.lower_ap(in_)]
    for v in (0.0, 1.0, 0.0):   # bias, scale, alpha
        ins.append(mybir.ImmediateValue(dtype=mybir.dt.float32, value=v))
    return eng.add_instruction(mybir.InstActivation(
        name=nc.get_next_instruction_name(),
        func=AF.Reciprocal, ins=ins, outs=[eng.lower_ap(out)]))

def _bcast_ap(ap, p=128):
    """Broadcast a 1-D DRAM vector across p partitions (step-0 partition dim)."""
    return bass.AP(tensor=ap.tensor, offset=ap.offset, ap=[[0, p]] + list(ap.ap))


def _emit(nc, apply_affine: bool):
    x_d = nc.dram_tensor("x", [L, C], f32, kind="ExternalInput").ap()
    wqkv_d = nc.dram_tensor("w_qkv", [C, 3 * C], f32r, kind="ExternalInput").ap()
    bqk_d = nc.dram_tensor("b_qk", [128, 16], f32, kind="ExternalInput").ap()
    wout_d = nc.dram_tensor("w_out", [C, C], bf16, kind="ExternalInput").ap()
    bout_d = nc.dram_tensor("b_out_eff", [C], f32, kind="ExternalInput").ap()
    if apply_affine:
        g_d = nc.dram_tensor("ln_g", [C], f32, kind="ExternalInput").ap()
        b_d = nc.dram_tensor("ln_b", [C], f32, kind="ExternalInput").ap()
    out_d = nc.dram_tensor("out", [L, C], f32, kind="ExternalOutput").ap()

    with nc.allow_low_precision(reason="bf16/f32r compute by design"), \
         tile.TileContext(nc) as tc, ExitStack() as ctx:
        # Long-lived pools on the LEFT side, allocation order chosen so that
        # the ones dying earliest are on top of the stack.
        const = ctx.enter_context(tc.tile_pool(name="const", bufs=1, side="left"))
        ident = const.tile([128, 128], f32)
        make_identity(nc, ident)
        ones_bf = const.tile([128, 1], bf16)
        nc.vector.memset(ones_bf, 1.0)
        eps_sb = const.tile([128, 1], f32)
        nc.vector.memset(eps_sb, EPS)
        bqk_sb = const.tile([128, 16], f32)
        nc.sync.dma_start(out=bqk_sb[:], in_=bqk_d)
        bv_d = nc.dram_tensor("b_v", [C], f32, kind="ExternalInput").ap()
        bv_bc = const.tile([128, C], f32)
        nc.gpsimd.dma_start(out=bv_bc[:], in_=_bcast_ap(bv_d))
        if apply_affine:
            g_bc = const.tile([128, C], f32)
            nc.gpsimd.dma_start(out=g_bc[:], in_=_bcast_ap(g_d))
            b_bc = const.tile([128, C], f32)
            nc.gpsimd.dma_start(out=b_bc[:], in_=_bcast_ap(b_d))

        xn_pool = ctx.enter_context(tc.tile_pool(name="xn", bufs=1, side="left"))
        xn = xn_pool.tile([128, NT, C], f32)     # normalized x, natural [l, c]
        attnT_pool = ctx.enter_context(tc.tile_pool(name="attnT", bufs=1, side="left"))
        attnT = attnT_pool.tile([128, NG, L], bf16)   # [c', g_q, l]
        v_pool = tc.alloc_tile_pool(name="v", bufs=1, side="left")
        v_bf = v_pool.tile([128, NT, C], bf16)   # [l_r, l-tile, c]

        # ---------------- Phase 1: LayerNorm ----------------
        with tc.tile_pool(name="xin", bufs=4, side="right") as xin, \
             tc.tile_pool(name="lnst", bufs=4, side="right") as lnst, \
             tc.tile_pool(name="lntmp", bufs=3, side="right") as lntmp:
            for t in range(NT):
                xt = xin.tile([128, C], f32)
                stats = lnst.tile([128, 2, 6], f32)
                for j in range(2):
                    nc.sync.dma_start(
                        out=xt[:, 512 * j:512 * (j + 1)],
                        in_=x_d[128 * t:128 * (t + 1), 512 * j:512 * (j + 1)])
                    nc.vector.bn_stats(out=stats[:, j, :],
                                       in_=xt[:, 512 * j:512 * (j + 1)])
                mv = lnst.tile([128, 2], f32)
                nc.vector.bn_aggr(out=mv[:], in_=stats[:])
                sq = lnst.tile([128, 1], f32)
                nc.scalar.activation(out=sq[:], in_=mv[:, 1:2], func=AF.Sqrt,
                                     bias=eps_sb[:], scale=1.0)
                rstd = lnst.tile([128, 1], f32)
                nc.vector.reciprocal(out=rstd[:], in_=sq[:])
                nmr = lnst.tile([128, 1], f32)
                nc.vector.tensor_scalar(nmr[:], mv[:, 0:1], rstd[:], -1.0,
                                        ALU.mult, ALU.mult)
                if apply_affine:
                    zt = lntmp.tile([128, C], f32)
                    nc.scalar.activation(out=zt[:], in_=xt[:], func=AF.Identity,
                                         bias=nmr[:], scale=rstd[:])
                    zg = lntmp.tile([128, C], f32)
                    nc.vector.tensor_tensor(out=zg[:], in0=zt[:], in1=g_bc[:],
                                            op=ALU.mult)
                    nc.vector.tensor_tensor(out=xn[:, t, :], in0=zg[:], in1=b_bc[:],
                                            op=ALU.add)
                else:
                    nc.scalar.activation(out=xn[:, t, :], in_=xt[:], func=AF.Identity,
                                         bias=nmr[:], scale=rstd[:])

        # ------- Phase 2-4: transpose xn -> xnT; V, Q, K projections -------
        with tc.tile_pool(name="xnT", bufs=1, side="right") as xnT_pool:
            xnT = xnT_pool.tile([128, NG, L], f32r)   # [c', g, l]
            with tc.tile_pool(name="tr_ps", bufs=4, space="PSUM") as tr_ps:
                for t in range(NT):
                    for g in range(NG):
                        ps = tr_ps.tile([128, 128], f32)
                        nc.tensor.transpose(ps[:], xn[:, t, 128 * g:128 * (g + 1)],
                                            ident[:])
                        nc.scalar.copy(out=xnT[:, g, 128 * t:128 * (t + 1)], in_=ps[:])

            with tc.tile_pool(name="proj_ps", bufs=3, space="PSUM") as proj_ps:
                # V projection (natural layout, bf16 out)
                with tc.tile_pool(name="wv", bufs=1, side="right") as wv_pool:
                    wv_sb = wv_pool.tile([128, NG, C], f32r)
                    nc.gpsimd.dma_start(
                        out=wv_sb[:],
                        in_=wqkv_d[:, 2 * C:3 * C].rearrange("(k p) n -> p k n", p=128))
                    for m in range(NT):
                        psv = proj_ps.tile([128, C], f32, tag="proj")
                        for ki in range(NG):
                            lhsT = xnT[:, ki, 128 * m:128 * (m + 1)]
                            for j in range(2):
                                nc.tensor.matmul(
                                    psv[:, 512 * j:512 * (j + 1)], lhsT,
                                    wv_sb[:, ki, 512 * j:512 * (j + 1)],
                                    start=(ki == 0), stop=(ki == NG - 1))
                        nc.vector.tensor_tensor(out=v_bf[:, m, :], in0=psv[:],
                                                in1=bv_bc[:], op=ALU.add)

                # Q, K projections (transposed layout)
                qT_pool = tc.alloc_tile_pool(name="qT", bufs=1, side="left")
                qT = qT_pool.tile([128, H, NG, 128], f32r)   # [c', h, g_q, l_r]
                kT_pool = tc.alloc_tile_pool(name="kT", bufs=1, side="left")
                kT = kT_pool.tile([128, NG, L], f32r)        # [c', g_k, l]
                with tc.tile_pool(name="wqk", bufs=6, side="right") as wqk_pool:
                    for co in range(16):
                        wslab = wqk_pool.tile([128, NG, 128], f32r)
                        nc.gpsimd.dma_start(
                            out=wslab[:],
                            in_=wqkv_d[:, 128 * co:128 * (co + 1)].rearrange(
                                "(k p) n -> p k n", p=128))
                        psq = proj_ps.tile([128, L], f32, tag="proj")
                        for ki in range(NG):
                            for j in range(2):
                                nc.tensor.matmul(
                                    psq[:, 512 * j:512 * (j + 1)],
                                    wslab[:, ki, :],
                                    xnT[:, ki, 512 * j:512 * (j + 1)],
                                    start=(ki == 0), stop=(ki == NG - 1))
                        bias_col = bqk_sb[:, co:co + 1]
                        if co < 8:
                            # q: dst [c', h, l_r] over h (l = 128h + l_r)
                            nc.vector.tensor_scalar(
                                qT[:, :, co, :],
                                psq[:].rearrange("p (h l) -> p h l", h=H),
                                bias_col, None, ALU.add)
                        else:
                            nc.vector.tensor_scalar(kT[:, co - 8, :], psq[:],
                                                    bias_col, None, ALU.add)

        # ---------------- Phase 5: attention ----------------
        pt_bufs = 1 if apply_affine else 2
        wout_pool = tc.alloc_tile_pool(name="wout", bufs=1, side="right")
        wout_sb = wout_pool.tile([128, NG, C], bf16)
        nc.gpsimd.dma_start(out=wout_sb[:],
                          in_=wout_d.rearrange("(k p) n -> p k n", p=128))
        with tc.tile_pool(name="pt", bufs=pt_bufs, side="right") as pt_pool, \
             tc.tile_pool(name="rb", bufs=2, side="right") as rb_pool, \
             tc.tile_pool(name="recip", bufs=2, side="right") as recip_pool, \
             tc.tile_pool(name="s_ps", bufs=2, space="PSUM", side="right") as s_ps, \
             tc.tile_pool(name="sum_ps", bufs=1, space="PSUM") as sum_ps, \
             tc.tile_pool(name="av_ps", bufs=1, space="PSUM") as av_ps:
            pend = []   # (h, pt, rb) awaiting attnV; emitted one head behind

            def emit_scores(h):
                pt = pt_pool.tile([128, NG, L], bf16, name=f"pt{h}", tag="pt")
                hs = slice(128 * h, 128 * (h + 1))
                ps_sum = sum_ps.tile([1, L], f32, tag="ps_sum")
                qrow = qT[:, h, :, :].rearrange("p g l -> p (g l)")
                for gk in range(NG):
                    ps_s = s_ps.tile([128, L], f32, tag="ps_s")
                    for j in range(2):
                        nc.tensor.matmul(ps_s[:, 512 * j:512 * (j + 1)],
                                         kT[:, gk, hs],
                                         qrow[:, 512 * j:512 * (j + 1)],
                                         start=True, stop=True)
                    nc.scalar.activation(out=pt[:, gk, :], in_=ps_s[:], func=AF.Exp,
                                         bias=0.0, scale=S2)
                    for j in range(2):
                        nc.tensor.matmul(ps_sum[:, 512 * j:512 * (j + 1)], ones_bf[:],
                                         pt[:, gk, 512 * j:512 * (j + 1)],
                                         start=(gk == 0), stop=(gk == NG - 1))
                recip = recip_pool.tile([1, L], f32, tag="recip")
                nc.vector.reciprocal_approx_fast(out=recip[:], in_=ps_sum[:])
                rb = rb_pool.tile([128, L], f32, tag="rb")
                nc.gpsimd.partition_broadcast(rb[:], recip[:])
                pend.append((h, pt, rb))

            def emit_attnv():
                h, pt, rb = pend.pop(0)
                hs = slice(128 * h, 128 * (h + 1))
                ps_av = av_ps.tile([128, L], f32, tag="ps_av")
                for gk in range(NG):
                    for j in range(2):
                        nc.tensor.matmul(ps_av[:, 512 * j:512 * (j + 1)],
                                         v_bf[:, h, 128 * gk:128 * (gk + 1)],
                                         pt[:, gk, 512 * j:512 * (j + 1)],
                                         start=(gk == 0), stop=(gk == NG - 1))
                # attnT[:, g_q, 128h + l_r] = ps_av[:, (g_q, l_r)] * rb
                nc.vector.tensor_tensor(
                    out=attnT[:, :, hs],
                    in0=ps_av[:].rearrange("p (g l) -> p g l", g=NG),
                    in1=rb[:].rearrange("p (g l) -> p g l", g=NG), op=ALU.mult)

            for h in range(H):
                emit_scores(h)
                if pend and h > 0:
                    emit_attnv()
            while pend:
                emit_attnv()

        kT_pool.release()
        qT_pool.release()
        v_pool.release()

        # ---------------- Phase 6: output projection + residual ----------------
        with tc.tile_pool(name="otile", bufs=4, side="right") as ot_pool, \
             tc.tile_pool(name="out_ps", bufs=2, space="PSUM") as out_ps:
            bout_bc = ot_pool.tile([128, C], f32)
            nc.gpsimd.dma_start(out=bout_bc[:], in_=_bcast_ap(bout_d))
            for m in range(NT):
                ps_o = out_ps.tile([128, C], f32)
                for ki in range(NG):
                    lhsT = attnT[:, ki, 128 * m:128 * (m + 1)]
                    for j in range(2):
                        nc.tensor.matmul(
                            ps_o[:, 512 * j:512 * (j + 1)], lhsT,
                            wout_sb[:, ki, 512 * j:512 * (j + 1)],
                            start=(ki == 0), stop=(ki == NG - 1))
                t1 = ot_pool.tile([128, C], f32)
                nc.vector.tensor_tensor(out=t1[:], in0=ps_o[:], in1=xn[:, m, :],
                                        op=ALU.add)
                t2 = ot_pool.tile([128, C], f32)
                nc.vector.tensor_tensor(out=t2[:], in0=t1[:], in1=bout_bc[:],
                                        op=ALU.add)
                nc.sync.dma_start(out=out_d[128 * m:128 * (m + 1), :], in_=t2[:])

        wout_pool.release()

    return nc


_CACHE = {}


def _build(apply_affine: bool):
    key = apply_affine
    if key not in _CACHE:
        nc = bacc.Bacc("TRN2", target_bir_lowering=False, debug=False)
        _emit(nc, apply_affine)
        nc.compile()
        _CACHE[key] = nc
    return _CACHE[key]


def kernel(**inputs) -> np.ndarray:
    x = np.asarray(inputs["x"], np.float32)
    ln_g = np.asarray(inputs["ln_g"], np.float32)
    ln_b = np.asarray(inputs["ln_b"], np.float32)
    w_qkv = np.ascontiguousarray(np.asarray(inputs["w_qkv"], np.float32))
    b_qkv = np.asarray(inputs["b_qkv"], np.float32)
    w_out = np.ascontiguousarray(np.asarray(inputs["w_out"], np.float32))
    b_out = np.asarray(inputs["b_out"], np.float32)

    B = x.shape[0]
    assert x.shape == (B, L, C)
    apply_affine = not (np.all(ln_g == 1.0) and np.all(ln_b == 0.0))
    nc = _build(apply_affine)

    b_out_eff = b_out
    bqk_pre = np.ascontiguousarray(b_qkv[:2 * C].reshape(16, 128).T)
    bv = np.ascontiguousarray(b_qkv[2 * C:])
    w_out_bf = w_out.astype(ml_dtypes.bfloat16)

    in_maps = []
    for c in range(B):
        m = {
            "x": np.ascontiguousarray(x[c]),
            "w_qkv": w_qkv,
            "b_qk": bqk_pre,
            "b_v": bv,
            "w_out": w_out_bf,
            "b_out_eff": b_out_eff,
        }
        if apply_affine:
            m["ln_g"] = ln_g
            m["ln_b"] = ln_b
        in_maps.append(m)

    res = bass_utils.run_bass_kernel_spmd(nc, in_maps, core_ids=list(range(B)))
    return np.stack([res.results[c]["out"] for c in range(B)]).astype(np.float32)


# revision 37
# speedup vs baseline: 1.1836x; 1.1836x over previous
"""Trainium2 Bass kernel for an AttentionBlock (LN -> QKV -> attn -> out-proj + residual).

Shapes (hardcoded per problem spec): B=8, L=1024, C=1024, H=8 heads.
The reference uses a raw row-major reshape (torch-style .view) of q/k/v from
[B, L, C] to [B*H, L, C/H]; with L=1024, C=1024, H=8 this makes each
"attention head" operate on a contiguous 128-sequence-row block of the
[L, C] matrix, reinterpreted as [1024, 128].

Sharding: pure data-parallel over batch, one batch element per NeuronCore
(8 cores). No collectives.
"""

import math
from contextlib import ExitStack

import ml_dtypes
import numpy as np

import concourse.bass as bass
import concourse.bacc as bacc
import concourse.tile as tile
from concourse import mybir
from concourse import bass_utils
from concourse.masks import make_identity

L = 1024
C = 1024
H = 8          # heads; also number of 128-row l-tiles (head h <-> l-tile h)
CH = 128       # head dim
NT = 8         # l tiles (128 rows each)
NG = 8         # c groups (128 cols each)
EPS = 1e-5
S2 = 1.0 / math.sqrt(CH)   # combined q&k scale: (ch^-0.25)^2

f32 = mybir.dt.float32
f32r = mybir.dt.float32r
bf16 = mybir.dt.bfloat16
AF = mybir.ActivationFunctionType
ALU = mybir.AluOpType



def _act_reciprocal(nc, out, in_):
    """Reciprocal on ScalarE (LUT). bass blocks this func due to accuracy
    concerns; for softmax denominators the ~1e-3 LUT error is negligible and
    the single-lane DVE reciprocal (6.5us for [1,1024]) is on the critical
    path."""
    eng = nc.scalar
    ins = [eng.lower_ap(in_)]
    for v in (0.0, 1.0, 0.0):   # bias, scale, alpha
        ins.append(mybir.ImmediateValue(dtype=mybir.dt.float32, value=v))
    return eng.add_instruction(mybir.InstActivation(
        name=nc.get_next_instruction_name(),
        func=AF.Reciprocal, ins=ins, outs=[eng.lower_ap(out)]))

def _bcast_ap(ap, p=128):
    """Broadcast a 1-D DRAM vector across p partitions (step-0 partition dim)."""
    return bass.AP(tensor=ap.tensor, offset=ap.offset, ap=[[0, p]] + list(ap.ap))


def _emit(nc, apply_affine: bool):
    x_d = nc.dram_tensor("x", [L, C], f32, kind="ExternalInput").ap()
    wqkv_d = nc.dram_tensor("w_qkv", [C, 3 * C], f32r, kind="ExternalInput").ap()
    bqk_d = nc.dram_tensor("b_qk", [128, 16], f32, kind="ExternalInput").ap()
    wout_d = nc.dram_tensor("w_out", [C, C], bf16, kind="ExternalInput").ap()
    bout_d = nc.dram_tensor("b_out_eff", [C], f32, kind="ExternalInput").ap()
    if apply_affine:
        g_d = nc.dram_tensor("ln_g", [C], f32, kind="ExternalInput").ap()
        b_d = nc.dram_tensor("ln_b", [C], f32, kind="ExternalInput").ap()
    out_d = nc.dram_tensor("out", [L, C], f32, kind="ExternalOutput").ap()

    with nc.allow_low_precision(reason="bf16/f32r compute by design"), \
         tile.TileContext(nc) as tc, ExitStack() as ctx:
        # Long-lived pools on the LEFT side, allocation order chosen so that
        # the ones dying earliest are on top of the stack.
        const = ctx.enter_context(tc.tile_pool(name="const", bufs=1, side="left"))
        ident = const.tile([128, 128], f32)
        make_identity(nc, ident)
        ones_bf = const.tile([128, 1], bf16)
        nc.vector.memset(ones_bf, 1.0)
        eps_sb = const.tile([128, 1], f32)
        nc.vector.memset(eps_sb, EPS)
        bqk_sb = const.tile([128, 16], f32)
        nc.sync.dma_start(out=bqk_sb[:], in_=bqk_d)
        bv_d = nc.dram_tensor("b_v", [C], f32, kind="ExternalInput").ap()
        bv_bc = const.tile([128, C], f32)
        nc.gpsimd.dma_start(out=bv_bc[:], in_=_bcast_ap(bv_d))
        if apply_affine:
            g_bc = const.tile([128, C], f32)
            nc.gpsimd.dma_start(out=g_bc[:], in_=_bcast_ap(g_d))
            b_bc = const.tile([128, C], f32)
            nc.gpsimd.dma_start(out=b_bc[:], in_=_bcast_ap(b_d))

        xn_pool = ctx.enter_context(tc.tile_pool(name="xn", bufs=1, side="left"))
        xn = xn_pool.tile([128, NT, C], f32)     # normalized x, natural [l, c]
        attnT_pool = ctx.enter_context(tc.tile_pool(name="attnT", bufs=1, side="left"))
        attnT = attnT_pool.tile([128, NG, L], bf16)   # [c', g_q, l]
        v_pool = tc.alloc_tile_pool(name="v", bufs=1, side="left")
        v_bf = v_pool.tile([128, NT, C], bf16)   # [l_r, l-tile, c]

        # ---------------- Phase 1: LayerNorm ----------------
        with tc.tile_pool(name="xin", bufs=4, side="right") as xin, \
             tc.tile_pool(name="lnst", bufs=4, side="right") as lnst, \
             tc.tile_pool(name="lntmp", bufs=3, side="right") as lntmp:
            for t in range(NT):
                xt = xin.tile([128, C], f32)
                stats = lnst.tile([128, 2, 6], f32)
                for j in range(2):
                    nc.sync.dma_start(
                        out=xt[:, 512 * j:512 * (j + 1)],
                        in_=x_d[128 * t:128 * (t + 1), 512 * j:512 * (j + 1)])
                    nc.vector.bn_stats(out=stats[:, j, :],
                                       in_=xt[:, 512 * j:512 * (j + 1)])
                mv = lnst.tile([128, 2], f32)
                nc.vector.bn_aggr(out=mv[:], in_=stats[:])
                sq = lnst.tile([128, 1], f32)
                nc.scalar.activation(out=sq[:], in_=mv[:, 1:2], func=AF.Sqrt,
                                     bias=eps_sb[:], scale=1.0)
                rstd = lnst.tile([128, 1], f32)
                nc.vector.reciprocal(out=rstd[:], in_=sq[:])
                nmr = lnst.tile([128, 1], f32)
                nc.vector.tensor_scalar(nmr[:], mv[:, 0:1], rstd[:], -1.0,
                                        ALU.mult, ALU.mult)
                if apply_affine:
                    zt = lntmp.tile([128, C], f32)
                    nc.scalar.activation(out=zt[:], in_=xt[:], func=AF.Identity,
                                         bias=nmr[:], scale=rstd[:])
                    zg = lntmp.tile([128, C], f32)
                    nc.vector.tensor_tensor(out=zg[:], in0=zt[:], in1=g_bc[:],
                                            op=ALU.mult)
                    nc.vector.tensor_tensor(out=xn[:, t, :], in0=zg[:], in1=b_bc[:],
                                            op=ALU.add)
                else:
                    nc.scalar.activation(out=xn[:, t, :], in_=xt[:], func=AF.Identity,
                                         bias=nmr[:], scale=rstd[:])

        # ------- Phase 2-4: transpose xn -> xnT; V, Q, K projections -------
        with tc.tile_pool(name="xnT", bufs=1, side="right") as xnT_pool:
            xnT = xnT_pool.tile([128, NG, L], f32r)   # [c', g, l]
            with tc.tile_pool(name="tr_ps", bufs=4, space="PSUM") as tr_ps:
                for t in range(NT):
                    for g in range(NG):
                        ps = tr_ps.tile([128, 128], f32)
                        nc.tensor.transpose(ps[:], xn[:, t, 128 * g:128 * (g + 1)],
                                            ident[:])
                        nc.scalar.copy(out=xnT[:, g, 128 * t:128 * (t + 1)], in_=ps[:])

            with tc.tile_pool(name="proj_ps", bufs=3, space="PSUM") as proj_ps:
                # V projection (natural layout, bf16 out)
                with tc.tile_pool(name="wv", bufs=1, side="right") as wv_pool:
                    wv_sb = wv_pool.tile([128, NG, C], f32r)
                    nc.sync.dma_start(
                        out=wv_sb[:],
                        in_=wqkv_d[:, 2 * C:3 * C].rearrange("(k p) n -> p k n", p=128))
                    for m in range(NT):
                        psv = proj_ps.tile([128, C], f32, tag="proj")
                        for ki in range(NG):
                            lhsT = xnT[:, ki, 128 * m:128 * (m + 1)]
                            for j in range(2):
                                nc.tensor.matmul(
                                    psv[:, 512 * j:512 * (j + 1)], lhsT,
                                    wv_sb[:, ki, 512 * j:512 * (j + 1)],
                                    start=(ki == 0), stop=(ki == NG - 1))
                        nc.vector.tensor_tensor(out=v_bf[:, m, :], in0=psv[:],
                                                in1=bv_bc[:], op=ALU.add)

                # Q, K projections (transposed layout)
                qT_pool = tc.alloc_tile_pool(name="qT", bufs=1, side="left")
                qT = qT_pool.tile([128, H, NG, 128], f32r)   # [c', h, g_q, l_r]
                kT_pool = tc.alloc_tile_pool(name="kT", bufs=1, side="left")
                kT = kT_pool.tile([128, NG, L], f32r)        # [c', g_k, l]
                with tc.tile_pool(name="wqk", bufs=6, side="right") as wqk_pool:
                    for co in range(16):
                        wslab = wqk_pool.tile([128, NG, 128], f32r)
                        nc.sync.dma_start(
                            out=wslab[:],
                            in_=wqkv_d[:, 128 * co:128 * (co + 1)].rearrange(
                                "(k p) n -> p k n", p=128))
                        psq = proj_ps.tile([128, L], f32, tag="proj")
                        for ki in range(NG):
                            for j in range(2):
                                nc.tensor.matmul(
                                    psq[:, 512 * j:512 * (j + 1)],
                                    wslab[:, ki, :],
                                    xnT[:, ki, 512 * j:512 * (j + 1)],
                                    start=(ki == 0), stop=(ki == NG - 1))
                        bias_col = bqk_sb[:, co:co + 1]
                        if co < 8:
                            # q: dst [c', h, l_r] over h (l = 128h + l_r)
                            nc.vector.tensor_scalar(
                                qT[:, :, co, :],
                                psq[:].rearrange("p (h l) -> p h l", h=H),
                                bias_col, None, ALU.add)
                        else:
                            nc.vector.tensor_scalar(kT[:, co - 8, :], psq[:],
                                                    bias_col, None, ALU.add)

        # ---------------- Phase 5: attention ----------------
        pt_bufs = 1 if apply_affine else 2
        wout_pool = tc.alloc_tile_pool(name="wout", bufs=1, side="right")
        wout_sb = wout_pool.tile([128, NG, C], bf16)
        nc.sync.dma_start(out=wout_sb[:],
                          in_=wout_d.rearrange("(k p) n -> p k n", p=128))
        with tc.tile_pool(name="pt", bufs=pt_bufs, side="right") as pt_pool, \
             tc.tile_pool(name="rb", bufs=2, side="right") as rb_pool, \
             tc.tile_pool(name="recip", bufs=2, side="right") as recip_pool, \
             tc.tile_pool(name="s_ps", bufs=2, space="PSUM", side="right") as s_ps, \
             tc.tile_pool(name="sum_ps", bufs=1, space="PSUM") as sum_ps, \
             tc.tile_pool(name="av_ps", bufs=1, space="PSUM") as av_ps:
            pend = []   # (h, pt, rb) awaiting attnV; emitted one head behind

            def emit_scores(h):
                pt = pt_pool.tile([128, NG, L], bf16, name=f"pt{h}", tag="pt")
                hs = slice(128 * h, 128 * (h + 1))
                ps_sum = sum_ps.tile([1, L], f32, tag="ps_sum")
                qrow = qT[:, h, :, :].rearrange("p g l -> p (g l)")
                for gk in range(NG):
                    ps_s = s_ps.tile([128, L], f32, tag="ps_s")
                    for j in range(2):
                        nc.tensor.matmul(ps_s[:, 512 * j:512 * (j + 1)],
                                         kT[:, gk, hs],
                                         qrow[:, 512 * j:512 * (j + 1)],
                                         start=True, stop=True)
                    nc.scalar.activation(out=pt[:, gk, :], in_=ps_s[:], func=AF.Exp,
                                         bias=0.0, scale=S2)
                    for j in range(2):
                        nc.tensor.matmul(ps_sum[:, 512 * j:512 * (j + 1)], ones_bf[:],
                                         pt[:, gk, 512 * j:512 * (j + 1)],
                                         start=(gk == 0), stop=(gk == NG - 1))
                recip = recip_pool.tile([1, L], f32, tag="recip")
                nc.vector.reciprocal_approx_fast(out=recip[:], in_=ps_sum[:])
                rb = rb_pool.tile([128, L], f32, tag="rb")
                nc.gpsimd.partition_broadcast(rb[:], recip[:])
                pend.append((h, pt, rb))

            def emit_attnv():
                h, pt, rb = pend.pop(0)
                hs = slice(128 * h, 128 * (h + 1))
                ps_av = av_ps.tile([128, L], f32, tag="ps_av")
                for gk in range(NG):
                    for j in range(2):
                        nc.tensor.matmul(ps_av[:, 512 * j:512 * (j + 1)],
                                         v_bf[:, h, 128 * gk:128 * (gk + 1)],
                                         pt[:, gk, 512 * j:512 * (j + 1)],
                                         start=(gk == 0), stop=(gk == NG - 1))
                # attnT[:, g_q, 128h + l_r] = ps_av[:, (g_q, l_r)] * rb
                nc.vector.tensor_tensor(
                    out=attnT[:, :, hs],
                    in0=ps_av[:].rearrange("p (g l) -> p g l", g=NG),
                    in1=rb[:].rearrange("p (g l) -> p g l", g=NG), op=ALU.mult)

            for h in range(H):
                emit_scores(h)
                if pend and h > 0:
                    emit_attnv()
            while pend:
                emit_attnv()

        kT_pool.release()
        qT_pool.release()
        v_pool.release()

        # ---------------- Phase 6: output projection + residual ----------------
        with tc.tile_pool(name="otile", bufs=4, side="right") as ot_pool, \
             tc.tile_pool(name="out_ps", bufs=2, space="PSUM") as out_ps:
            bout_bc = ot_pool.tile([128, C], f32)
            nc.gpsimd.dma_start(out=bout_bc[:], in_=_bcast_ap(bout_d))
            for m in range(NT):
                ps_o = out_ps.tile([128, C], f32)
                for ki in range(NG):
                    lhsT = attnT[:, ki, 128 * m:128 * (m + 1)]
                    for j in range(2):
                        nc.tensor.matmul(
                            ps_o[:, 512 * j:512 * (j + 1)], lhsT,
                            wout_sb[:, ki, 512 * j:512 * (j + 1)],
                            start=(ki == 0), stop=(ki == NG - 1))
                t1 = ot_pool.tile([128, C], f32)
                nc.vector.tensor_tensor(out=t1[:], in0=ps_o[:], in1=xn[:, m, :],
                                        op=ALU.add)
                t2 = ot_pool.tile([128, C], f32)
                nc.vector.tensor_tensor(out=t2[:], in0=t1[:], in1=bout_bc[:],
                                        op=ALU.add)
                nc.sync.dma_start(out=out_d[128 * m:128 * (m + 1), :], in_=t2[:])

        wout_pool.release()

    return nc


_CACHE = {}


def _build(apply_affine: bool):
    key = apply_affine
    if key not in _CACHE:
        nc = bacc.Bacc("TRN2", target_bir_lowering=False, debug=False)
        _emit(nc, apply_affine)
        nc.compile()
        _CACHE[key] = nc
    return _CACHE[key]


def kernel(**inputs) -> np.ndarray:
    x = np.asarray(inputs["x"], np.float32)
    ln_g = np.asarray(inputs["ln_g"], np.float32)
    ln_b = np.asarray(inputs["ln_b"], np.float32)
    w_qkv = np.ascontiguousarray(np.asarray(inputs["w_qkv"], np.float32))
    b_qkv = np.asarray(inputs["b_qkv"], np.float32)
    w_out = np.ascontiguousarray(np.asarray(inputs["w_out"], np.float32))
    b_out = np.asarray(inputs["b_out"], np.float32)

    B = x.shape[0]
    assert x.shape == (B, L, C)
    apply_affine = not (np.all(ln_g == 1.0) and np.all(ln_b == 0.0))
    nc = _build(apply_affine)

    b_out_eff = b_out
    bqk_pre = np.ascontiguousarray(b_qkv[:2 * C].reshape(16, 128).T)
    bv = np.ascontiguousarray(b_qkv[2 * C:])
    w_out_bf = w_out.astype(ml_dtypes.bfloat16)

    in_maps = []
    for c in range(B):
        m = {
            "x": np.ascontiguousarray(x[c]),
            "w_qkv": w_qkv,
            "b_qk": bqk_pre,
            "b_v": bv,
            "w_out": w_out_bf,
            "b_out_eff": b_out_eff,
        }
        if apply_affine:
            m["ln_g"] = ln_g
            m["ln_b"] = ln_b
        in_maps.append(m)

    res = bass_utils.run_bass_kernel_spmd(nc, in_maps, core_ids=list(range(B)))
    return np.stack([res.results[c]["out"] for c in range(B)]).astype(np.float32)


# revision 38
# speedup vs baseline: 1.1841x; 1.0004x over previous
"""Trainium2 Bass kernel for an AttentionBlock (LN -> QKV -> attn -> out-proj + residual).

Shapes (hardcoded per problem spec): B=8, L=1024, C=1024, H=8 heads.
The reference uses a raw row-major reshape (torch-style .view) of q/k/v from
[B, L, C] to [B*H, L, C/H]; with L=1024, C=1024, H=8 this makes each
"attention head" operate on a contiguous 128-sequence-row block of the
[L, C] matrix, reinterpreted as [1024, 128].

Sharding: pure data-parallel over batch, one batch element per NeuronCore
(8 cores). No collectives.
"""

import math
from contextlib import ExitStack

import ml_dtypes
import numpy as np

import concourse.bass as bass
import concourse.bacc as bacc
import concourse.tile as tile
from concourse import mybir
from concourse import bass_utils
from concourse.masks import make_identity

L = 1024
C = 1024
H = 8          # heads; also number of 128-row l-tiles (head h <-> l-tile h)
CH = 128       # head dim
NT = 8         # l tiles (128 rows each)
NG = 8         # c groups (128 cols each)
EPS = 1e-5
S2 = 1.0 / math.sqrt(CH)   # combined q&k scale: (ch^-0.25)^2

f32 = mybir.dt.float32
f32r = mybir.dt.float32r
bf16 = mybir.dt.bfloat16
AF = mybir.ActivationFunctionType
ALU = mybir.AluOpType



def _bcast_ap(ap, p=128):
    """Broadcast a 1-D DRAM vector across p partitions (step-0 partition dim)."""
    return bass.AP(tensor=ap.tensor, offset=ap.offset, ap=[[0, p]] + list(ap.ap))


def _emit(nc, apply_affine: bool):
    x_d = nc.dram_tensor("x", [L, C], f32, kind="ExternalInput").ap()
    wqkv_d = nc.dram_tensor("w_qkv", [C, 3 * C], f32r, kind="ExternalInput").ap()
    bqk_d = nc.dram_tensor("b_qk", [128, 16], f32, kind="ExternalInput").ap()
    wout_d = nc.dram_tensor("w_out", [C, C], bf16, kind="ExternalInput").ap()
    bout_d = nc.dram_tensor("b_out_eff", [C], f32, kind="ExternalInput").ap()
    if apply_affine:
        g_d = nc.dram_tensor("ln_g", [C], f32, kind="ExternalInput").ap()
        b_d = nc.dram_tensor("ln_b", [C], f32, kind="ExternalInput").ap()
    out_d = nc.dram_tensor("out", [L, C], f32, kind="ExternalOutput").ap()

    with nc.allow_low_precision(reason="bf16/f32r compute by design"), \
         tile.TileContext(nc) as tc, ExitStack() as ctx:
        # Long-lived pools on the LEFT side, allocation order chosen so that
        # the ones dying earliest are on top of the stack.
        const = ctx.enter_context(tc.tile_pool(name="const", bufs=1, side="left"))
        ident = const.tile([128, 128], f32)
        make_identity(nc, ident)
        ones_bf = const.tile([128, 1], bf16)
        nc.vector.memset(ones_bf, 1.0)
        eps_sb = const.tile([128, 1], f32)
        nc.vector.memset(eps_sb, EPS)
        bqk_sb = const.tile([128, 16], f32)
        nc.sync.dma_start(out=bqk_sb[:], in_=bqk_d)
        bv_d = nc.dram_tensor("b_v", [C], f32, kind="ExternalInput").ap()
        bv_bc = const.tile([128, C], f32)
        nc.gpsimd.dma_start(out=bv_bc[:], in_=_bcast_ap(bv_d))
        if apply_affine:
            g_bc = const.tile([128, C], f32)
            nc.gpsimd.dma_start(out=g_bc[:], in_=_bcast_ap(g_d))
            b_bc = const.tile([128, C], f32)
            nc.gpsimd.dma_start(out=b_bc[:], in_=_bcast_ap(b_d))

        xn_pool = ctx.enter_context(tc.tile_pool(name="xn", bufs=1, side="left"))
        xn = xn_pool.tile([128, NT, C], f32)     # normalized x, natural [l, c]
        attnT_pool = ctx.enter_context(tc.tile_pool(name="attnT", bufs=1, side="left"))
        attnT = attnT_pool.tile([128, NG, L], bf16)   # [c', g_q, l]
        v_pool = tc.alloc_tile_pool(name="v", bufs=1, side="left")
        v_bf = v_pool.tile([128, NT, C], bf16)   # [l_r, l-tile, c]

        # ---------------- Phase 1: LayerNorm ----------------
        with tc.tile_pool(name="xin", bufs=4, side="right") as xin, \
             tc.tile_pool(name="lnst", bufs=4, side="right") as lnst, \
             tc.tile_pool(name="lntmp", bufs=3, side="right") as lntmp:
            for t in range(NT):
                xt = xin.tile([128, C], f32)
                stats = lnst.tile([128, 2, 6], f32)
                for j in range(2):
                    nc.sync.dma_start(
                        out=xt[:, 512 * j:512 * (j + 1)],
                        in_=x_d[128 * t:128 * (t + 1), 512 * j:512 * (j + 1)])
                    nc.vector.bn_stats(out=stats[:, j, :],
                                       in_=xt[:, 512 * j:512 * (j + 1)])
                mv = lnst.tile([128, 2], f32)
                nc.vector.bn_aggr(out=mv[:], in_=stats[:])
                sq = lnst.tile([128, 1], f32)
                nc.scalar.activation(out=sq[:], in_=mv[:, 1:2], func=AF.Sqrt,
                                     bias=eps_sb[:], scale=1.0)
                rstd = lnst.tile([128, 1], f32)
                nc.vector.reciprocal(out=rstd[:], in_=sq[:])
                nmr = lnst.tile([128, 1], f32)
                nc.vector.tensor_scalar(nmr[:], mv[:, 0:1], rstd[:], -1.0,
                                        ALU.mult, ALU.mult)
                if apply_affine:
                    zt = lntmp.tile([128, C], f32)
                    nc.scalar.activation(out=zt[:], in_=xt[:], func=AF.Identity,
                                         bias=nmr[:], scale=rstd[:])
                    zg = lntmp.tile([128, C], f32)
                    nc.vector.tensor_tensor(out=zg[:], in0=zt[:], in1=g_bc[:],
                                            op=ALU.mult)
                    nc.vector.tensor_tensor(out=xn[:, t, :], in0=zg[:], in1=b_bc[:],
                                            op=ALU.add)
                else:
                    nc.scalar.activation(out=xn[:, t, :], in_=xt[:], func=AF.Identity,
                                         bias=nmr[:], scale=rstd[:])

        # ------- Phase 2-4: transpose xn -> xnT; V, Q, K projections -------
        with tc.tile_pool(name="xnT", bufs=1, side="right") as xnT_pool:
            xnT = xnT_pool.tile([128, NG, L], f32r)   # [c', g, l]
            with tc.tile_pool(name="tr_ps", bufs=4, space="PSUM") as tr_ps:
                for t in range(NT):
                    for g in range(NG):
                        ps = tr_ps.tile([128, 128], f32)
                        nc.tensor.transpose(ps[:], xn[:, t, 128 * g:128 * (g + 1)],
                                            ident[:])
                        nc.scalar.copy(out=xnT[:, g, 128 * t:128 * (t + 1)], in_=ps[:])

            with tc.tile_pool(name="proj_ps", bufs=3, space="PSUM") as proj_ps:
                # V projection (natural layout, bf16 out)
                with tc.tile_pool(name="wv", bufs=1, side="right") as wv_pool:
                    wv_sb = wv_pool.tile([128, NG, C], f32r)
                    nc.sync.dma_start(
                        out=wv_sb[:],
                        in_=wqkv_d[:, 2 * C:3 * C].rearrange("(k p) n -> p k n", p=128))
                    for m in range(NT):
                        psv = proj_ps.tile([128, C], f32, tag="proj")
                        for ki in range(NG):
                            lhsT = xnT[:, ki, 128 * m:128 * (m + 1)]
                            for j in range(2):
                                nc.tensor.matmul(
                                    psv[:, 512 * j:512 * (j + 1)], lhsT,
                                    wv_sb[:, ki, 512 * j:512 * (j + 1)],
                                    start=(ki == 0), stop=(ki == NG - 1))
                        nc.vector.tensor_tensor(out=v_bf[:, m, :], in0=psv[:],
                                                in1=bv_bc[:], op=ALU.add)

                # Q, K projections (transposed layout)
                qT_pool = tc.alloc_tile_pool(name="qT", bufs=1, side="left")
                qT = qT_pool.tile([128, H, NG, 128], f32r)   # [c', h, g_q, l_r]
                kT_pool = tc.alloc_tile_pool(name="kT", bufs=1, side="left")
                kT = kT_pool.tile([128, NG, L], f32r)        # [c', g_k, l]
                with tc.tile_pool(name="wqk", bufs=6, side="right") as wqk_pool:
                    for co in range(16):
                        wslab = wqk_pool.tile([128, NG, 128], f32r)
                        nc.sync.dma_start(
                            out=wslab[:],
                            in_=wqkv_d[:, 128 * co:128 * (co + 1)].rearrange(
                                "(k p) n -> p k n", p=128))
                        psq = proj_ps.tile([128, L], f32, tag="proj")
                        for ki in range(NG):
                            for j in range(2):
                                nc.tensor.matmul(
                                    psq[:, 512 * j:512 * (j + 1)],
                                    wslab[:, ki, :],
                                    xnT[:, ki, 512 * j:512 * (j + 1)],
                                    start=(ki == 0), stop=(ki == NG - 1))
                        bias_col = bqk_sb[:, co:co + 1]
                        if co < 8:
                            # q: dst [c', h, l_r] over h (l = 128h + l_r)
                            nc.vector.tensor_scalar(
                                qT[:, :, co, :],
                                psq[:].rearrange("p (h l) -> p h l", h=H),
                                bias_col, None, ALU.add)
                        else:
                            nc.vector.tensor_scalar(kT[:, co - 8, :], psq[:],
                                                    bias_col, None, ALU.add)

        # ---------------- Phase 5: attention ----------------
        pt_bufs = 1 if apply_affine else 2
        wout_pool = tc.alloc_tile_pool(name="wout", bufs=1, side="right")
        wout_sb = wout_pool.tile([128, NG, C], bf16)
        nc.sync.dma_start(out=wout_sb[:],
                          in_=wout_d.rearrange("(k p) n -> p k n", p=128))
        with tc.tile_pool(name="pt", bufs=pt_bufs, side="right") as pt_pool, \
             tc.tile_pool(name="rb", bufs=2, side="right") as rb_pool, \
             tc.tile_pool(name="recip", bufs=2, side="right") as recip_pool, \
             tc.tile_pool(name="s_ps", bufs=2, space="PSUM", side="right") as s_ps, \
             tc.tile_pool(name="sum_ps", bufs=1, space="PSUM") as sum_ps, \
             tc.tile_pool(name="av_ps", bufs=1, space="PSUM") as av_ps:
            pend = []   # (h, pt, rb) awaiting attnV; emitted one head behind

            def emit_scores(h):
                pt = pt_pool.tile([128, NG, L], bf16, name=f"pt{h}", tag="pt")
                hs = slice(128 * h, 128 * (h + 1))
                ps_sum = sum_ps.tile([1, L], f32, tag="ps_sum")
                qrow = qT[:, h, :, :].rearrange("p g l -> p (g l)")
                for gk in range(NG):
                    ps_s = s_ps.tile([128, L], f32, tag="ps_s")
                    for j in range(2):
                        nc.tensor.matmul(ps_s[:, 512 * j:512 * (j + 1)],
                                         kT[:, gk, hs],
                                         qrow[:, 512 * j:512 * (j + 1)],
                                         start=True, stop=True)
                    nc.scalar.activation(out=pt[:, gk, :], in_=ps_s[:], func=AF.Exp,
                                         bias=0.0, scale=S2)
                    for j in range(2):
                        nc.tensor.matmul(ps_sum[:, 512 * j:512 * (j + 1)], ones_bf[:],
                                         pt[:, gk, 512 * j:512 * (j + 1)],
                                         start=(gk == 0), stop=(gk == NG - 1))
                recip = recip_pool.tile([1, L], f32, tag="recip")
                nc.vector.reciprocal_approx_fast(out=recip[:], in_=ps_sum[:])
                rb = rb_pool.tile([128, L], f32, tag="rb")
                nc.gpsimd.partition_broadcast(rb[:], recip[:])
                pend.append((h, pt, rb))

            def emit_attnv():
                h, pt, rb = pend.pop(0)
                hs = slice(128 * h, 128 * (h + 1))
                ps_av = av_ps.tile([128, L], f32, tag="ps_av")
                for gk in range(NG):
                    for j in range(2):
                        nc.tensor.matmul(ps_av[:, 512 * j:512 * (j + 1)],
                                         v_bf[:, h, 128 * gk:128 * (gk + 1)],
                                         pt[:, gk, 512 * j:512 * (j + 1)],
                                         start=(gk == 0), stop=(gk == NG - 1))
                # attnT[:, g_q, 128h + l_r] = ps_av[:, (g_q, l_r)] * rb
                nc.vector.tensor_tensor(
                    out=attnT[:, :, hs],
                    in0=ps_av[:].rearrange("p (g l) -> p g l", g=NG),
                    in1=rb[:].rearrange("p (g l) -> p g l", g=NG), op=ALU.mult)

            for h in range(H):
                emit_scores(h)
                if pend and h > 0:
                    emit_attnv()
            while pend:
                emit_attnv()

        kT_pool.release()
        qT_pool.release()
        v_pool.release()

        # ---------------- Phase 6: output projection + residual ----------------
        with tc.tile_pool(name="otile", bufs=4, side="right") as ot_pool, \
             tc.tile_pool(name="out_ps", bufs=2, space="PSUM") as out_ps:
            bout_bc = ot_pool.tile([128, C], f32)
            nc.gpsimd.dma_start(out=bout_bc[:], in_=_bcast_ap(bout_d))
            for m in range(NT):
                ps_o = out_ps.tile([128, C], f32)
                for ki in range(NG):
                    lhsT = attnT[:, ki, 128 * m:128 * (m + 1)]
                    for j in range(2):
                        nc.tensor.matmul(
                            ps_o[:, 512 * j:512 * (j + 1)], lhsT,
                            wout_sb[:, ki, 512 * j:512 * (j + 1)],
                            start=(ki == 0), stop=(ki == NG - 1))
                t1 = ot_pool.tile([128, C], f32)
                nc.vector.tensor_tensor(out=t1[:], in0=ps_o[:], in1=xn[:, m, :],
                                        op=ALU.add)
                t2 = ot_pool.tile([128, C], f32)
                nc.vector.tensor_tensor(out=t2[:], in0=t1[:], in1=bout_bc[:],
                                        op=ALU.add)
                nc.sync.dma_start(out=out_d[128 * m:128 * (m + 1), :], in_=t2[:])

        wout_pool.release()

    return nc


_CACHE = {}


def _build(apply_affine: bool):
    key = apply_affine
    if key not in _CACHE:
        nc = bacc.Bacc("TRN2", target_bir_lowering=False, debug=False)
        _emit(nc, apply_affine)
        nc.compile()
        _CACHE[key] = nc
    return _CACHE[key]


def kernel(**inputs) -> np.ndarray:
    x = np.asarray(inputs["x"], np.float32)
    ln_g = np.asarray(inputs["ln_g"], np.float32)
    ln_b = np.asarray(inputs["ln_b"], np.float32)
    w_qkv = np.ascontiguousarray(np.asarray(inputs["w_qkv"], np.float32))
    b_qkv = np.asarray(inputs["b_qkv"], np.float32)
    w_out = np.ascontiguousarray(np.asarray(inputs["w_out"], np.float32))
    b_out = np.asarray(inputs["b_out"], np.float32)

    B = x.shape[0]
    assert x.shape == (B, L, C)
    apply_affine = not (np.all(ln_g == 1.0) and np.all(ln_b == 0.0))
    nc = _build(apply_affine)

    b_out_eff = b_out
    bqk_pre = np.ascontiguousarray(b_qkv[:2 * C].reshape(16, 128).T)
    bv = np.ascontiguousarray(b_qkv[2 * C:])
    w_out_bf = w_out.astype(ml_dtypes.bfloat16)

    in_maps = []
    for c in range(B):
        m = {
            "x": np.ascontiguousarray(x[c]),
            "w_qkv": w_qkv,
            "b_qk": bqk_pre,
            "b_v": bv,
            "w_out": w_out_bf,
            "b_out_eff": b_out_eff,
        }
        if apply_affine:
            m["ln_g"] = ln_g
            m["ln_b"] = ln_b
        in_maps.append(m)

    res = bass_utils.run_bass_kernel_spmd(nc, in_maps, core_ids=list(range(B)))
    return np.stack([res.results[c]["out"] for c in range(B)]).astype(np.float32)


# revision 39
# speedup vs baseline: 1.2213x; 1.0315x over previous
"""Trainium2 Bass kernel for an AttentionBlock (LN -> QKV -> attn -> out-proj + residual).

Shapes (hardcoded per problem spec): B=8, L=1024, C=1024, H=8 heads.
The reference uses a raw row-major reshape (torch-style .view) of q/k/v from
[B, L, C] to [B*H, L, C/H]; with L=1024, C=1024, H=8 this makes each
"attention head" operate on a contiguous 128-sequence-row block of the
[L, C] matrix, reinterpreted as [1024, 128].

Sharding: pure data-parallel over batch, one batch element per NeuronCore
(8 cores). No collectives.
"""

import math
from contextlib import ExitStack

import ml_dtypes
import numpy as np

import concourse.bass as bass
import concourse.bacc as bacc
import concourse.tile as tile
from concourse import mybir
from concourse import bass_utils
from concourse.masks import make_identity

L = 1024
C = 1024
H = 8          # heads; also number of 128-row l-tiles (head h <-> l-tile h)
CH = 128       # head dim
NT = 8         # l tiles (128 rows each)
NG = 8         # c groups (128 cols each)
EPS = 1e-5
S2 = 1.0 / math.sqrt(CH)   # combined q&k scale: (ch^-0.25)^2

f32 = mybir.dt.float32
f32r = mybir.dt.float32r
bf16 = mybir.dt.bfloat16
AF = mybir.ActivationFunctionType
ALU = mybir.AluOpType



def _bcast_ap(ap, p=128):
    """Broadcast a 1-D DRAM vector across p partitions (step-0 partition dim)."""
    return bass.AP(tensor=ap.tensor, offset=ap.offset, ap=[[0, p]] + list(ap.ap))


def _emit(nc, apply_affine: bool):
    x_d = nc.dram_tensor("x", [L, C], f32, kind="ExternalInput").ap()
    wqkv_d = nc.dram_tensor("w_qkv", [C, 3 * C], f32r, kind="ExternalInput").ap()
    bqk_d = nc.dram_tensor("b_qk", [128, 16], f32, kind="ExternalInput").ap()
    wout_d = nc.dram_tensor("w_out", [C, C], bf16, kind="ExternalInput").ap()
    bout_d = nc.dram_tensor("b_out_eff", [C], f32, kind="ExternalInput").ap()
    if apply_affine:
        g_d = nc.dram_tensor("ln_g", [C], f32, kind="ExternalInput").ap()
        b_d = nc.dram_tensor("ln_b", [C], f32, kind="ExternalInput").ap()
    out_d = nc.dram_tensor("out", [L, C], f32, kind="ExternalOutput").ap()

    with nc.allow_low_precision(reason="bf16/f32r compute by design"), \
         tile.TileContext(nc) as tc, ExitStack() as ctx:
        # Long-lived pools on the LEFT side, allocation order chosen so that
        # the ones dying earliest are on top of the stack.
        const = ctx.enter_context(tc.tile_pool(name="const", bufs=1, side="left"))
        ident = const.tile([128, 128], f32)
        make_identity(nc, ident)
        ones_bf = const.tile([128, 1], bf16)
        nc.vector.memset(ones_bf, 1.0)
        eps_sb = const.tile([128, 1], f32)
        nc.vector.memset(eps_sb, EPS)
        bqk_sb = const.tile([128, 16], f32)
        nc.sync.dma_start(out=bqk_sb[:], in_=bqk_d)
        bv_d = nc.dram_tensor("b_v", [C], f32, kind="ExternalInput").ap()
        bv_bc = const.tile([128, C], f32)
        nc.gpsimd.dma_start(out=bv_bc[:], in_=_bcast_ap(bv_d))
        if apply_affine:
            g_bc = const.tile([128, C], f32)
            nc.gpsimd.dma_start(out=g_bc[:], in_=_bcast_ap(g_d))
            b_bc = const.tile([128, C], f32)
            nc.gpsimd.dma_start(out=b_bc[:], in_=_bcast_ap(b_d))

        xn_pool = ctx.enter_context(tc.tile_pool(name="xn", bufs=1, side="left"))
        xn = xn_pool.tile([128, NT, C], f32)     # normalized x, natural [l, c]
        attnT_pool = ctx.enter_context(tc.tile_pool(name="attnT", bufs=1, side="left"))
        attnT = attnT_pool.tile([128, NG, L], bf16)   # [c', g_q, l]
        v_pool = tc.alloc_tile_pool(name="v", bufs=1, side="left")
        v_bf = v_pool.tile([128, NT, C], bf16)   # [l_r, l-tile, c]
        wv_pool = tc.alloc_tile_pool(name="wv", bufs=1, side="left")
        wv_sb = wv_pool.tile([128, NG, C], f32r)

        # ---------------- Phase 1: LayerNorm ----------------
        with tc.tile_pool(name="xin", bufs=4, side="right") as xin, \
             tc.tile_pool(name="lnst", bufs=4, side="right") as lnst, \
             tc.tile_pool(name="lntmp", bufs=3, side="right") as lntmp:
            for t in range(NT):
                xt = xin.tile([128, C], f32)
                stats = lnst.tile([128, 2, 6], f32)
                for j in range(2):
                    nc.sync.dma_start(
                        out=xt[:, 512 * j:512 * (j + 1)],
                        in_=x_d[128 * t:128 * (t + 1), 512 * j:512 * (j + 1)])
                    nc.vector.bn_stats(out=stats[:, j, :],
                                       in_=xt[:, 512 * j:512 * (j + 1)])
                mv = lnst.tile([128, 2], f32)
                nc.vector.bn_aggr(out=mv[:], in_=stats[:])
                sq = lnst.tile([128, 1], f32)
                nc.scalar.activation(out=sq[:], in_=mv[:, 1:2], func=AF.Sqrt,
                                     bias=eps_sb[:], scale=1.0)
                rstd = lnst.tile([128, 1], f32)
                nc.vector.reciprocal(out=rstd[:], in_=sq[:])
                nmr = lnst.tile([128, 1], f32)
                nc.vector.tensor_scalar(nmr[:], mv[:, 0:1], rstd[:], -1.0,
                                        ALU.mult, ALU.mult)
                if apply_affine:
                    zt = lntmp.tile([128, C], f32)
                    nc.scalar.activation(out=zt[:], in_=xt[:], func=AF.Identity,
                                         bias=nmr[:], scale=rstd[:])
                    zg = lntmp.tile([128, C], f32)
                    nc.vector.tensor_tensor(out=zg[:], in0=zt[:], in1=g_bc[:],
                                            op=ALU.mult)
                    nc.vector.tensor_tensor(out=xn[:, t, :], in0=zg[:], in1=b_bc[:],
                                            op=ALU.add)
                else:
                    nc.scalar.activation(out=xn[:, t, :], in_=xt[:], func=AF.Identity,
                                         bias=nmr[:], scale=rstd[:])

        # ------- Phase 2-4: transpose xn -> xnT; V, Q, K projections -------
        nc.sync.dma_start(
            out=wv_sb[:],
            in_=wqkv_d[:, 2 * C:3 * C].rearrange("(k p) n -> p k n", p=128))
        with tc.tile_pool(name="xnT", bufs=1, side="right") as xnT_pool:
            xnT = xnT_pool.tile([128, NG, L], f32r)   # [c', g, l]
            with tc.tile_pool(name="tr_ps", bufs=4, space="PSUM") as tr_ps:
                for t in range(NT):
                    for g in range(NG):
                        ps = tr_ps.tile([128, 128], f32)
                        nc.tensor.transpose(ps[:], xn[:, t, 128 * g:128 * (g + 1)],
                                            ident[:])
                        nc.scalar.copy(out=xnT[:, g, 128 * t:128 * (t + 1)], in_=ps[:])

            with tc.tile_pool(name="proj_ps", bufs=3, space="PSUM") as proj_ps:
                # V projection (natural layout, bf16 out)
                for m in range(NT):
                    psv = proj_ps.tile([128, C], f32, tag="proj")
                    for ki in range(NG):
                        lhsT = xnT[:, ki, 128 * m:128 * (m + 1)]
                        for j in range(2):
                            nc.tensor.matmul(
                                psv[:, 512 * j:512 * (j + 1)], lhsT,
                                wv_sb[:, ki, 512 * j:512 * (j + 1)],
                                start=(ki == 0), stop=(ki == NG - 1))
                    nc.vector.tensor_tensor(out=v_bf[:, m, :], in0=psv[:],
                                            in1=bv_bc[:], op=ALU.add)
                wv_pool.release()

                # Q, K projections (transposed layout)
                qT_pool = tc.alloc_tile_pool(name="qT", bufs=1, side="left")
                qT = qT_pool.tile([128, H, NG, 128], f32r)   # [c', h, g_q, l_r]
                kT_pool = tc.alloc_tile_pool(name="kT", bufs=1, side="left")
                kT = kT_pool.tile([128, NG, L], f32r)        # [c', g_k, l]
                with tc.tile_pool(name="wqk", bufs=6, side="right") as wqk_pool:
                    for co in range(16):
                        wslab = wqk_pool.tile([128, NG, 128], f32r)
                        nc.sync.dma_start(
                            out=wslab[:],
                            in_=wqkv_d[:, 128 * co:128 * (co + 1)].rearrange(
                                "(k p) n -> p k n", p=128))
                        psq = proj_ps.tile([128, L], f32, tag="proj")
                        for ki in range(NG):
                            for j in range(2):
                                nc.tensor.matmul(
                                    psq[:, 512 * j:512 * (j + 1)],
                                    wslab[:, ki, :],
                                    xnT[:, ki, 512 * j:512 * (j + 1)],
                                    start=(ki == 0), stop=(ki == NG - 1))
                        bias_col = bqk_sb[:, co:co + 1]
                        if co < 8:
                            # q: dst [c', h, l_r] over h (l = 128h + l_r)
                            nc.vector.tensor_scalar(
                                qT[:, :, co, :],
                                psq[:].rearrange("p (h l) -> p h l", h=H),
                                bias_col, None, ALU.add)
                        else:
                            nc.vector.tensor_scalar(kT[:, co - 8, :], psq[:],
                                                    bias_col, None, ALU.add)

        # ---------------- Phase 5: attention ----------------
        pt_bufs = 1 if apply_affine else 2
        wout_pool = tc.alloc_tile_pool(name="wout", bufs=1, side="right")
        wout_sb = wout_pool.tile([128, NG, C], bf16)
        nc.sync.dma_start(out=wout_sb[:],
                          in_=wout_d.rearrange("(k p) n -> p k n", p=128))
        with tc.tile_pool(name="pt", bufs=pt_bufs, side="right") as pt_pool, \
             tc.tile_pool(name="rb", bufs=2, side="right") as rb_pool, \
             tc.tile_pool(name="recip", bufs=2, side="right") as recip_pool, \
             tc.tile_pool(name="s_ps", bufs=2, space="PSUM", side="right") as s_ps, \
             tc.tile_pool(name="sum_ps", bufs=1, space="PSUM") as sum_ps, \
             tc.tile_pool(name="av_ps", bufs=1, space="PSUM") as av_ps:
            pend = []   # (h, pt, rb) awaiting attnV; emitted one head behind

            def emit_scores(h):
                pt = pt_pool.tile([128, NG, L], bf16, name=f"pt{h}", tag="pt")
                hs = slice(128 * h, 128 * (h + 1))
                ps_sum = sum_ps.tile([1, L], f32, tag="ps_sum")
                qrow = qT[:, h, :, :].rearrange("p g l -> p (g l)")
                for gk in range(NG):
                    ps_s = s_ps.tile([128, L], f32, tag="ps_s")
                    for j in range(2):
                        nc.tensor.matmul(ps_s[:, 512 * j:512 * (j + 1)],
                                         kT[:, gk, hs],
                                         qrow[:, 512 * j:512 * (j + 1)],
                                         start=True, stop=True)
                    nc.scalar.activation(out=pt[:, gk, :], in_=ps_s[:], func=AF.Exp,
                                         bias=0.0, scale=S2)
                    for j in range(2):
                        nc.tensor.matmul(ps_sum[:, 512 * j:512 * (j + 1)], ones_bf[:],
                                         pt[:, gk, 512 * j:512 * (j + 1)],
                                         start=(gk == 0), stop=(gk == NG - 1))
                recip = recip_pool.tile([1, L], f32, tag="recip")
                nc.vector.reciprocal_approx_fast(out=recip[:], in_=ps_sum[:])
                rb = rb_pool.tile([128, L], f32, tag="rb")
                nc.gpsimd.partition_broadcast(rb[:], recip[:])
                pend.append((h, pt, rb))

            def emit_attnv():
                h, pt, rb = pend.pop(0)
                hs = slice(128 * h, 128 * (h + 1))
                ps_av = av_ps.tile([128, L], f32, tag="ps_av")
                for gk in range(NG):
                    for j in range(2):
                        nc.tensor.matmul(ps_av[:, 512 * j:512 * (j + 1)],
                                         v_bf[:, h, 128 * gk:128 * (gk + 1)],
                                         pt[:, gk, 512 * j:512 * (j + 1)],
                                         start=(gk == 0), stop=(gk == NG - 1))
                # attnT[:, g_q, 128h + l_r] = ps_av[:, (g_q, l_r)] * rb
                nc.vector.tensor_tensor(
                    out=attnT[:, :, hs],
                    in0=ps_av[:].rearrange("p (g l) -> p g l", g=NG),
                    in1=rb[:].rearrange("p (g l) -> p g l", g=NG), op=ALU.mult)

            for h in range(H):
                emit_scores(h)
                if pend and h > 0:
                    emit_attnv()
            while pend:
                emit_attnv()

        kT_pool.release()
        qT_pool.release()
        v_pool.release()

        # ---------------- Phase 6: output projection + residual ----------------
        with tc.tile_pool(name="otile", bufs=4, side="right") as ot_pool, \
             tc.tile_pool(name="out_ps", bufs=2, space="PSUM") as out_ps:
            bout_bc = ot_pool.tile([128, C], f32)
            nc.gpsimd.dma_start(out=bout_bc[:], in_=_bcast_ap(bout_d))
            for m in range(NT):
                ps_o = out_ps.tile([128, C], f32)
                for ki in range(NG):
                    lhsT = attnT[:, ki, 128 * m:128 * (m + 1)]
                    for j in range(2):
                        nc.tensor.matmul(
                            ps_o[:, 512 * j:512 * (j + 1)], lhsT,
                            wout_sb[:, ki, 512 * j:512 * (j + 1)],
                            start=(ki == 0), stop=(ki == NG - 1))
                t1 = ot_pool.tile([128, C], f32)
                nc.vector.tensor_tensor(out=t1[:], in0=ps_o[:], in1=xn[:, m, :],
                                        op=ALU.add)
                t2 = ot_pool.tile([128, C], f32)
                nc.vector.tensor_tensor(out=t2[:], in0=t1[:], in1=bout_bc[:],
                                        op=ALU.add)
                nc.sync.dma_start(out=out_d[128 * m:128 * (m + 1), :], in_=t2[:])

        wout_pool.release()

    return nc


_CACHE = {}


def _build(apply_affine: bool):
    key = apply_affine
    if key not in _CACHE:
        nc = bacc.Bacc("TRN2", target_bir_lowering=False, debug=False)
        _emit(nc, apply_affine)
        nc.compile()
        _CACHE[key] = nc
    return _CACHE[key]


def kernel(**inputs) -> np.ndarray:
    x = np.asarray(inputs["x"], np.float32)
    ln_g = np.asarray(inputs["ln_g"], np.float32)
    ln_b = np.asarray(inputs["ln_b"], np.float32)
    w_qkv = np.ascontiguousarray(np.asarray(inputs["w_qkv"], np.float32))
    b_qkv = np.asarray(inputs["b_qkv"], np.float32)
    w_out = np.ascontiguousarray(np.asarray(inputs["w_out"], np.float32))
    b_out = np.asarray(inputs["b_out"], np.float32)

    B = x.shape[0]
    assert x.shape == (B, L, C)
    apply_affine = not (np.all(ln_g == 1.0) and np.all(ln_b == 0.0))
    nc = _build(apply_affine)

    b_out_eff = b_out
    bqk_pre = np.ascontiguousarray(b_qkv[:2 * C].reshape(16, 128).T)
    bv = np.ascontiguousarray(b_qkv[2 * C:])
    w_out_bf = w_out.astype(ml_dtypes.bfloat16)

    in_maps = []
    for c in range(B):
        m = {
            "x": np.ascontiguousarray(x[c]),
            "w_qkv": w_qkv,
            "b_qk": bqk_pre,
            "b_v": bv,
            "w_out": w_out_bf,
            "b_out_eff": b_out_eff,
        }
        if apply_affine:
            m["ln_g"] = ln_g
            m["ln_b"] = ln_b
        in_maps.append(m)

    res = bass_utils.run_bass_kernel_spmd(nc, in_maps, core_ids=list(range(B)))
    return np.stack([res.results[c]["out"] for c in range(B)]).astype(np.float32)


# revision 40
# speedup vs baseline: 1.2279x; 1.0053x over previous
"""Trainium2 Bass kernel for an AttentionBlock (LN -> QKV -> attn -> out-proj + residual).

Shapes (hardcoded per problem spec): B=8, L=1024, C=1024, H=8 heads.
The reference uses a raw row-major reshape (torch-style .view) of q/k/v from
[B, L, C] to [B*H, L, C/H]; with L=1024, C=1024, H=8 this makes each
"attention head" operate on a contiguous 128-sequence-row block of the
[L, C] matrix, reinterpreted as [1024, 128].

Sharding: pure data-parallel over batch, one batch element per NeuronCore
(8 cores). No collectives.
"""

import math
from contextlib import ExitStack

import ml_dtypes
import numpy as np

import concourse.bass as bass
import concourse.bacc as bacc
import concourse.tile as tile
from concourse import mybir
from concourse import bass_utils
from concourse.masks import make_identity

L = 1024
C = 1024
H = 8          # heads; also number of 128-row l-tiles (head h <-> l-tile h)
CH = 128       # head dim
NT = 8         # l tiles (128 rows each)
NG = 8         # c groups (128 cols each)
EPS = 1e-5
S2 = 1.0 / math.sqrt(CH)   # combined q&k scale: (ch^-0.25)^2

f32 = mybir.dt.float32
f32r = mybir.dt.float32r
bf16 = mybir.dt.bfloat16
AF = mybir.ActivationFunctionType
ALU = mybir.AluOpType



def _bcast_ap(ap, p=128):
    """Broadcast a 1-D DRAM vector across p partitions (step-0 partition dim)."""
    return bass.AP(tensor=ap.tensor, offset=ap.offset, ap=[[0, p]] + list(ap.ap))


def _emit(nc, apply_affine: bool):
    x_d = nc.dram_tensor("x", [L, C], f32, kind="ExternalInput").ap()
    wqkv_d = nc.dram_tensor("w_qkv", [C, 3 * C], f32r, kind="ExternalInput").ap()
    bqk_d = nc.dram_tensor("b_qk", [128, 16], f32, kind="ExternalInput").ap()
    wout_d = nc.dram_tensor("w_out", [C, C], bf16, kind="ExternalInput").ap()
    bout_d = nc.dram_tensor("b_out_eff", [C], f32, kind="ExternalInput").ap()
    if apply_affine:
        g_d = nc.dram_tensor("ln_g", [C], f32, kind="ExternalInput").ap()
        b_d = nc.dram_tensor("ln_b", [C], f32, kind="ExternalInput").ap()
    out_d = nc.dram_tensor("out", [L, C], f32, kind="ExternalOutput").ap()

    with nc.allow_low_precision(reason="bf16/f32r compute by design"), \
         tile.TileContext(nc) as tc, ExitStack() as ctx:
        # Long-lived pools on the LEFT side, allocation order chosen so that
        # the ones dying earliest are on top of the stack.
        const = ctx.enter_context(tc.tile_pool(name="const", bufs=1, side="left"))
        ident = const.tile([128, 128], f32)
        make_identity(nc, ident)
        ones_bf = const.tile([128, 1], bf16)
        nc.vector.memset(ones_bf, 1.0)
        eps_sb = const.tile([128, 1], f32)
        nc.vector.memset(eps_sb, EPS)
        bqk_sb = const.tile([128, 16], f32)
        nc.sync.dma_start(out=bqk_sb[:], in_=bqk_d)
        bv_d = nc.dram_tensor("b_v", [C], f32, kind="ExternalInput").ap()
        bv_bc = const.tile([128, C], f32)
        nc.gpsimd.dma_start(out=bv_bc[:], in_=_bcast_ap(bv_d))
        if apply_affine:
            g_bc = const.tile([128, C], f32)
            nc.gpsimd.dma_start(out=g_bc[:], in_=_bcast_ap(g_d))
            b_bc = const.tile([128, C], f32)
            nc.gpsimd.dma_start(out=b_bc[:], in_=_bcast_ap(b_d))

        xn_pool = ctx.enter_context(tc.tile_pool(name="xn", bufs=1, side="left"))
        xn = xn_pool.tile([128, NT, C], f32)     # normalized x, natural [l, c]
        attnT_pool = ctx.enter_context(tc.tile_pool(name="attnT", bufs=1, side="left"))
        attnT = attnT_pool.tile([128, NG, L], bf16)   # [c', g_q, l]
        v_pool = tc.alloc_tile_pool(name="v", bufs=1, side="left")
        v_bf = v_pool.tile([128, NT, C], bf16)   # [l_r, l-tile, c]
        wv_pool = tc.alloc_tile_pool(name="wv", bufs=1, side="left")
        wv_sb = wv_pool.tile([128, NG, C], f32r)

        # ---------------- Phase 1: LayerNorm ----------------
        with tc.tile_pool(name="xin", bufs=4, side="right") as xin, \
             tc.tile_pool(name="lnst", bufs=4, side="right") as lnst, \
             tc.tile_pool(name="lntmp", bufs=3, side="right") as lntmp:
            for t in range(NT):
                xt = xin.tile([128, C], f32)
                stats = lnst.tile([128, 2, 6], f32)
                for j in range(2):
                    nc.sync.dma_start(
                        out=xt[:, 512 * j:512 * (j + 1)],
                        in_=x_d[128 * t:128 * (t + 1), 512 * j:512 * (j + 1)])
                    nc.vector.bn_stats(out=stats[:, j, :],
                                       in_=xt[:, 512 * j:512 * (j + 1)])
                mv = lnst.tile([128, 2], f32)
                nc.vector.bn_aggr(out=mv[:], in_=stats[:])
                sq = lnst.tile([128, 1], f32)
                nc.scalar.activation(out=sq[:], in_=mv[:, 1:2], func=AF.Sqrt,
                                     bias=eps_sb[:], scale=1.0)
                rstd = lnst.tile([128, 1], f32)
                nc.vector.reciprocal(out=rstd[:], in_=sq[:])
                nmr = lnst.tile([128, 1], f32)
                nc.vector.tensor_scalar(nmr[:], mv[:, 0:1], rstd[:], -1.0,
                                        ALU.mult, ALU.mult)
                if apply_affine:
                    zt = lntmp.tile([128, C], f32)
                    nc.scalar.activation(out=zt[:], in_=xt[:], func=AF.Identity,
                                         bias=nmr[:], scale=rstd[:])
                    zg = lntmp.tile([128, C], f32)
                    nc.vector.tensor_tensor(out=zg[:], in0=zt[:], in1=g_bc[:],
                                            op=ALU.mult)
                    nc.vector.tensor_tensor(out=xn[:, t, :], in0=zg[:], in1=b_bc[:],
                                            op=ALU.add)
                else:
                    nc.scalar.activation(out=xn[:, t, :], in_=xt[:], func=AF.Identity,
                                         bias=nmr[:], scale=rstd[:])

        # ------- Phase 2-4: transpose xn -> xnT; V, Q, K projections -------
        nc.sync.dma_start(
            out=wv_sb[:],
            in_=wqkv_d[:, 2 * C:3 * C].rearrange("(k p) n -> p k n", p=128))
        with tc.tile_pool(name="xnT", bufs=1, side="right") as xnT_pool:
            xnT = xnT_pool.tile([128, NG, L], f32r)   # [c', g, l]
            with tc.tile_pool(name="tr_ps", bufs=4, space="PSUM") as tr_ps:
                for t in range(NT):
                    for g in range(NG):
                        ps = tr_ps.tile([128, 128], f32)
                        nc.tensor.transpose(ps[:], xn[:, t, 128 * g:128 * (g + 1)],
                                            ident[:])
                        nc.scalar.copy(out=xnT[:, g, 128 * t:128 * (t + 1)], in_=ps[:])

            with tc.tile_pool(name="proj_ps", bufs=3, space="PSUM") as proj_ps:
                # V projection (natural layout, bf16 out)
                for m in range(NT):
                    psv = proj_ps.tile([128, C], f32, tag="proj")
                    for ki in range(NG):
                        lhsT = xnT[:, ki, 128 * m:128 * (m + 1)]
                        for j in range(2):
                            nc.tensor.matmul(
                                psv[:, 512 * j:512 * (j + 1)], lhsT,
                                wv_sb[:, ki, 512 * j:512 * (j + 1)],
                                start=(ki == 0), stop=(ki == NG - 1))
                    nc.vector.tensor_tensor(out=v_bf[:, m, :], in0=psv[:],
                                            in1=bv_bc[:], op=ALU.add)
                wv_pool.release()

                # Q, K projections (transposed layout)
                qT_pool = tc.alloc_tile_pool(name="qT", bufs=1, side="left")
                qT = qT_pool.tile([128, H, NG, 128], bf16)   # [c', h, g_q, l_r]
                kT_pool = tc.alloc_tile_pool(name="kT", bufs=1, side="left")
                kT = kT_pool.tile([128, NG, L], bf16)        # [c', g_k, l]
                with tc.tile_pool(name="wqk", bufs=6, side="right") as wqk_pool:
                    for co in range(16):
                        wslab = wqk_pool.tile([128, NG, 128], f32r)
                        nc.sync.dma_start(
                            out=wslab[:],
                            in_=wqkv_d[:, 128 * co:128 * (co + 1)].rearrange(
                                "(k p) n -> p k n", p=128))
                        psq = proj_ps.tile([128, L], f32, tag="proj")
                        for ki in range(NG):
                            for j in range(2):
                                nc.tensor.matmul(
                                    psq[:, 512 * j:512 * (j + 1)],
                                    wslab[:, ki, :],
                                    xnT[:, ki, 512 * j:512 * (j + 1)],
                                    start=(ki == 0), stop=(ki == NG - 1))
                        bias_col = bqk_sb[:, co:co + 1]
                        if co < 8:
                            # q: dst [c', h, l_r] over h (l = 128h + l_r)
                            nc.vector.tensor_scalar(
                                qT[:, :, co, :],
                                psq[:].rearrange("p (h l) -> p h l", h=H),
                                bias_col, None, ALU.add)
                        else:
                            nc.vector.tensor_scalar(kT[:, co - 8, :], psq[:],
                                                    bias_col, None, ALU.add)

        # ---------------- Phase 5: attention ----------------
        pt_bufs = 1 if apply_affine else 2
        wout_pool = tc.alloc_tile_pool(name="wout", bufs=1, side="right")
        wout_sb = wout_pool.tile([128, NG, C], bf16)
        nc.sync.dma_start(out=wout_sb[:],
                          in_=wout_d.rearrange("(k p) n -> p k n", p=128))
        with tc.tile_pool(name="pt", bufs=pt_bufs, side="right") as pt_pool, \
             tc.tile_pool(name="rb", bufs=2, side="right") as rb_pool, \
             tc.tile_pool(name="recip", bufs=2, side="right") as recip_pool, \
             tc.tile_pool(name="s_ps", bufs=2, space="PSUM", side="right") as s_ps, \
             tc.tile_pool(name="sum_ps", bufs=1, space="PSUM") as sum_ps, \
             tc.tile_pool(name="av_ps", bufs=1, space="PSUM") as av_ps:
            pend = []   # (h, pt, rb) awaiting attnV; emitted one head behind

            def emit_scores(h):
                pt = pt_pool.tile([128, NG, L], bf16, name=f"pt{h}", tag="pt")
                hs = slice(128 * h, 128 * (h + 1))
                ps_sum = sum_ps.tile([1, L], f32, tag="ps_sum")
                qrow = qT[:, h, :, :].rearrange("p g l -> p (g l)")
                for gk in range(NG):
                    ps_s = s_ps.tile([128, L], f32, tag="ps_s")
                    for j in range(2):
                        nc.tensor.matmul(ps_s[:, 512 * j:512 * (j + 1)],
                                         kT[:, gk, hs],
                                         qrow[:, 512 * j:512 * (j + 1)],
                                         start=True, stop=True)
                    nc.scalar.activation(out=pt[:, gk, :], in_=ps_s[:], func=AF.Exp,
                                         bias=0.0, scale=S2)
                    for j in range(2):
                        nc.tensor.matmul(ps_sum[:, 512 * j:512 * (j + 1)], ones_bf[:],
                                         pt[:, gk, 512 * j:512 * (j + 1)],
                                         start=(gk == 0), stop=(gk == NG - 1))
                recip = recip_pool.tile([1, L], f32, tag="recip")
                nc.vector.reciprocal_approx_fast(out=recip[:], in_=ps_sum[:])
                rb = rb_pool.tile([128, L], f32, tag="rb")
                nc.gpsimd.partition_broadcast(rb[:], recip[:])
                pend.append((h, pt, rb))

            def emit_attnv():
                h, pt, rb = pend.pop(0)
                hs = slice(128 * h, 128 * (h + 1))
                ps_av = av_ps.tile([128, L], f32, tag="ps_av")
                for gk in range(NG):
                    for j in range(2):
                        nc.tensor.matmul(ps_av[:, 512 * j:512 * (j + 1)],
                                         v_bf[:, h, 128 * gk:128 * (gk + 1)],
                                         pt[:, gk, 512 * j:512 * (j + 1)],
                                         start=(gk == 0), stop=(gk == NG - 1))
                # attnT[:, g_q, 128h + l_r] = ps_av[:, (g_q, l_r)] * rb
                nc.vector.tensor_tensor(
                    out=attnT[:, :, hs],
                    in0=ps_av[:].rearrange("p (g l) -> p g l", g=NG),
                    in1=rb[:].rearrange("p (g l) -> p g l", g=NG), op=ALU.mult)

            for h in range(H):
                emit_scores(h)
                if pend and h > 0:
                    emit_attnv()
            while pend:
                emit_attnv()

        kT_pool.release()
        qT_pool.release()
        v_pool.release()

        # ---------------- Phase 6: output projection + residual ----------------
        with tc.tile_pool(name="otile", bufs=4, side="right") as ot_pool, \
             tc.tile_pool(name="out_ps", bufs=2, space="PSUM") as out_ps:
            bout_bc = ot_pool.tile([128, C], f32)
            nc.gpsimd.dma_start(out=bout_bc[:], in_=_bcast_ap(bout_d))
            for m in range(NT):
                ps_o = out_ps.tile([128, C], f32)
                for ki in range(NG):
                    lhsT = attnT[:, ki, 128 * m:128 * (m + 1)]
                    for j in range(2):
                        nc.tensor.matmul(
                            ps_o[:, 512 * j:512 * (j + 1)], lhsT,
                            wout_sb[:, ki, 512 * j:512 * (j + 1)],
                            start=(ki == 0), stop=(ki == NG - 1))
                t1 = ot_pool.tile([128, C], f32)
                nc.vector.tensor_tensor(out=t1[:], in0=ps_o[:], in1=xn[:, m, :],
                                        op=ALU.add)
                t2 = ot_pool.tile([128, C], f32)
                nc.vector.tensor_tensor(out=t2[:], in0=t1[:], in1=bout_bc[:],
                                        op=ALU.add)
                nc.sync.dma_start(out=out_d[128 * m:128 * (m + 1), :], in_=t2[:])

        wout_pool.release()

    return nc


_CACHE = {}


def _build(apply_affine: bool):
    key = apply_affine
    if key not in _CACHE:
        nc = bacc.Bacc("TRN2", target_bir_lowering=False, debug=False)
        _emit(nc, apply_affine)
        nc.compile()
        _CACHE[key] = nc
    return _CACHE[key]


def kernel(**inputs) -> np.ndarray:
    x = np.asarray(inputs["x"], np.float32)
    ln_g = np.asarray(inputs["ln_g"], np.float32)
    ln_b = np.asarray(inputs["ln_b"], np.float32)
    w_qkv = np.ascontiguousarray(np.asarray(inputs["w_qkv"], np.float32))
    b_qkv = np.asarray(inputs["b_qkv"], np.float32)
    w_out = np.ascontiguousarray(np.asarray(inputs["w_out"], np.float32))
    b_out = np.asarray(inputs["b_out"], np.float32)

    B = x.shape[0]
    assert x.shape == (B, L, C)
    apply_affine = not (np.all(ln_g == 1.0) and np.all(ln_b == 0.0))
    nc = _build(apply_affine)

    b_out_eff = b_out
    bqk_pre = np.ascontiguousarray(b_qkv[:2 * C].reshape(16, 128).T)
    bv = np.ascontiguousarray(b_qkv[2 * C:])
    w_out_bf = w_out.astype(ml_dtypes.bfloat16)

    in_maps = []
    for c in range(B):
        m = {
            "x": np.ascontiguousarray(x[c]),
            "w_qkv": w_qkv,
            "b_qk": bqk_pre,
            "b_v": bv,
            "w_out": w_out_bf,
            "b_out_eff": b_out_eff,
        }
        if apply_affine:
            m["ln_g"] = ln_g
            m["ln_b"] = ln_b
        in_maps.append(m)

    res = bass_utils.run_bass_kernel_spmd(nc, in_maps, core_ids=list(range(B)))
    return np.stack([res.results[c]["out"] for c in range(B)]).astype(np.float32)


# revision 41
# speedup vs baseline: 1.2396x; 1.0095x over previous
"""Trainium2 Bass kernel for an AttentionBlock (LN -> QKV -> attn -> out-proj + residual).

Shapes (hardcoded per problem spec): B=8, L=1024, C=1024, H=8 heads.
The reference uses a raw row-major reshape (torch-style .view) of q/k/v from
[B, L, C] to [B*H, L, C/H]; with L=1024, C=1024, H=8 this makes each
"attention head" operate on a contiguous 128-sequence-row block of the
[L, C] matrix, reinterpreted as [1024, 128].

Sharding: pure data-parallel over batch, one batch element per NeuronCore
(8 cores). No collectives.
"""

import math
from contextlib import ExitStack

import ml_dtypes
import numpy as np

import concourse.bass as bass
import concourse.bacc as bacc
import concourse.tile as tile
from concourse import mybir
from concourse import bass_utils
from concourse.masks import make_identity

L = 1024
C = 1024
H = 8          # heads; also number of 128-row l-tiles (head h <-> l-tile h)
CH = 128       # head dim
NT = 8         # l tiles (128 rows each)
NG = 8         # c groups (128 cols each)
EPS = 1e-5
S2 = 1.0 / math.sqrt(CH)   # combined q&k scale: (ch^-0.25)^2

f32 = mybir.dt.float32
f32r = mybir.dt.float32r
bf16 = mybir.dt.bfloat16
AF = mybir.ActivationFunctionType
ALU = mybir.AluOpType



def _bcast_ap(ap, p=128):
    """Broadcast a 1-D DRAM vector across p partitions (step-0 partition dim)."""
    return bass.AP(tensor=ap.tensor, offset=ap.offset, ap=[[0, p]] + list(ap.ap))


def _emit(nc, apply_affine: bool):
    x_d = nc.dram_tensor("x", [L, C], f32, kind="ExternalInput").ap()
    wqkv_d = nc.dram_tensor("w_qkv", [C, 3 * C], f32r, kind="ExternalInput").ap()
    bqk_d = nc.dram_tensor("b_qk", [128, 16], f32, kind="ExternalInput").ap()
    wout_d = nc.dram_tensor("w_out", [C, C], bf16, kind="ExternalInput").ap()
    bout_d = nc.dram_tensor("b_out_eff", [C], f32, kind="ExternalInput").ap()
    if apply_affine:
        g_d = nc.dram_tensor("ln_g", [C], f32, kind="ExternalInput").ap()
        b_d = nc.dram_tensor("ln_b", [C], f32, kind="ExternalInput").ap()
    out_d = nc.dram_tensor("out", [L, C], f32, kind="ExternalOutput").ap()

    with nc.allow_low_precision(reason="bf16/f32r compute by design"), \
         tile.TileContext(nc) as tc, ExitStack() as ctx:
        # Long-lived pools on the LEFT side, allocation order chosen so that
        # the ones dying earliest are on top of the stack.
        const = ctx.enter_context(tc.tile_pool(name="const", bufs=1, side="left"))
        ident = const.tile([128, 128], f32)
        make_identity(nc, ident)
        ones_bf = const.tile([128, 1], bf16)
        nc.vector.memset(ones_bf, 1.0)
        eps_sb = const.tile([128, 1], f32)
        nc.vector.memset(eps_sb, EPS)
        bqk_sb = const.tile([128, 16], f32)
        nc.sync.dma_start(out=bqk_sb[:], in_=bqk_d)
        bv_d = nc.dram_tensor("b_v", [C], f32, kind="ExternalInput").ap()
        bv_bc = const.tile([128, C], f32)
        nc.gpsimd.dma_start(out=bv_bc[:], in_=_bcast_ap(bv_d))
        if apply_affine:
            g_bc = const.tile([128, C], f32)
            nc.gpsimd.dma_start(out=g_bc[:], in_=_bcast_ap(g_d))
            b_bc = const.tile([128, C], f32)
            nc.gpsimd.dma_start(out=b_bc[:], in_=_bcast_ap(b_d))

        xn_pool = ctx.enter_context(tc.tile_pool(name="xn", bufs=1, side="left"))
        xn = xn_pool.tile([128, NT, C], f32)     # normalized x, natural [l, c]
        attnT_pool = ctx.enter_context(tc.tile_pool(name="attnT", bufs=1, side="left"))
        attnT = attnT_pool.tile([128, NG, L], bf16)   # [c', g_q, l]
        v_pool = tc.alloc_tile_pool(name="v", bufs=1, side="left")
        v_bf = v_pool.tile([128, NT, C], bf16)   # [l_r, l-tile, c]
        wv_pool = tc.alloc_tile_pool(name="wv", bufs=1, side="left")
        wv_sb = wv_pool.tile([128, NG, C], f32r)

        # ---------------- Phase 1: LayerNorm ----------------
        with tc.tile_pool(name="xin", bufs=4, side="right") as xin, \
             tc.tile_pool(name="lnst", bufs=4, side="right") as lnst, \
             tc.tile_pool(name="lntmp", bufs=3, side="right") as lntmp:
            for t in range(NT):
                xt = xin.tile([128, C], f32)
                stats = lnst.tile([128, 2, 6], f32)
                for j in range(2):
                    nc.sync.dma_start(
                        out=xt[:, 512 * j:512 * (j + 1)],
                        in_=x_d[128 * t:128 * (t + 1), 512 * j:512 * (j + 1)])
                    nc.vector.bn_stats(out=stats[:, j, :],
                                       in_=xt[:, 512 * j:512 * (j + 1)])
                mv = lnst.tile([128, 2], f32)
                nc.vector.bn_aggr(out=mv[:], in_=stats[:])
                sq = lnst.tile([128, 1], f32)
                nc.scalar.activation(out=sq[:], in_=mv[:, 1:2], func=AF.Sqrt,
                                     bias=eps_sb[:], scale=1.0)
                rstd = lnst.tile([128, 1], f32)
                nc.vector.reciprocal(out=rstd[:], in_=sq[:])
                nmr = lnst.tile([128, 1], f32)
                nc.vector.tensor_scalar(nmr[:], mv[:, 0:1], rstd[:], -1.0,
                                        ALU.mult, ALU.mult)
                if apply_affine:
                    zt = lntmp.tile([128, C], f32)
                    nc.scalar.activation(out=zt[:], in_=xt[:], func=AF.Identity,
                                         bias=nmr[:], scale=rstd[:])
                    zg = lntmp.tile([128, C], f32)
                    nc.vector.tensor_tensor(out=zg[:], in0=zt[:], in1=g_bc[:],
                                            op=ALU.mult)
                    nc.vector.tensor_tensor(out=xn[:, t, :], in0=zg[:], in1=b_bc[:],
                                            op=ALU.add)
                else:
                    nc.scalar.activation(out=xn[:, t, :], in_=xt[:], func=AF.Identity,
                                         bias=nmr[:], scale=rstd[:])

        # ------- Phase 2-4: transpose xn -> xnT; V, Q, K projections -------
        nc.sync.dma_start(
            out=wv_sb[:],
            in_=wqkv_d[:, 2 * C:3 * C].rearrange("(k p) n -> p k n", p=128))
        with tc.tile_pool(name="xnT", bufs=1, side="right") as xnT_pool:
            xnT = xnT_pool.tile([128, NG, L], f32r)   # [c', g, l]
            with tc.tile_pool(name="tr_ps", bufs=4, space="PSUM") as tr_ps:
                for t in range(NT):
                    for g in range(NG):
                        ps = tr_ps.tile([128, 128], f32)
                        nc.tensor.transpose(ps[:], xn[:, t, 128 * g:128 * (g + 1)],
                                            ident[:])
                        nc.scalar.copy(out=xnT[:, g, 128 * t:128 * (t + 1)], in_=ps[:])

            with tc.tile_pool(name="proj_ps", bufs=3, space="PSUM") as proj_ps:
                # V projection (natural layout, bf16 out)
                for m in range(NT):
                    psv = proj_ps.tile([128, C], f32, tag="proj")
                    for ki in range(NG):
                        lhsT = xnT[:, ki, 128 * m:128 * (m + 1)]
                        for j in range(2):
                            nc.tensor.matmul(
                                psv[:, 512 * j:512 * (j + 1)], lhsT,
                                wv_sb[:, ki, 512 * j:512 * (j + 1)],
                                start=(ki == 0), stop=(ki == NG - 1))
                    nc.vector.tensor_tensor(out=v_bf[:, m, :], in0=psv[:],
                                            in1=bv_bc[:], op=ALU.add)
                wv_pool.release()

                # Q, K projections (transposed layout)
                qT_pool = tc.alloc_tile_pool(name="qT", bufs=1, side="left")
                qT = qT_pool.tile([128, H, NG, 128], bf16)   # [c', h, g_q, l_r]
                kT_pool = tc.alloc_tile_pool(name="kT", bufs=1, side="left")
                kT = kT_pool.tile([128, NG, L], bf16)        # [c', g_k, l]
                with tc.tile_pool(name="wqk", bufs=6, side="right") as wqk_pool:
                    for co in range(16):
                        wslab = wqk_pool.tile([128, NG, 128], f32r)
                        nc.sync.dma_start(
                            out=wslab[:],
                            in_=wqkv_d[:, 128 * co:128 * (co + 1)].rearrange(
                                "(k p) n -> p k n", p=128))
                        psq = proj_ps.tile([128, L], f32, tag="proj")
                        for ki in range(NG):
                            for j in range(2):
                                nc.tensor.matmul(
                                    psq[:, 512 * j:512 * (j + 1)],
                                    wslab[:, ki, :],
                                    xnT[:, ki, 512 * j:512 * (j + 1)],
                                    start=(ki == 0), stop=(ki == NG - 1))
                        bias_col = bqk_sb[:, co:co + 1]
                        if co < 8:
                            # q: dst [c', h, l_r] over h (l = 128h + l_r)
                            nc.vector.tensor_scalar(
                                qT[:, :, co, :],
                                psq[:].rearrange("p (h l) -> p h l", h=H),
                                bias_col, None, ALU.add)
                        else:
                            nc.vector.tensor_scalar(kT[:, co - 8, :], psq[:],
                                                    bias_col, None, ALU.add)

        # ---------------- Phase 5: attention ----------------
        pt_bufs = 1 if apply_affine else 2
        wout_pool = tc.alloc_tile_pool(name="wout", bufs=1, side="right")
        wout_sb = wout_pool.tile([128, NG, C], bf16)
        nc.sync.dma_start(out=wout_sb[:],
                          in_=wout_d.rearrange("(k p) n -> p k n", p=128))
        with tc.tile_pool(name="pt", bufs=pt_bufs, side="right") as pt_pool, \
             tc.tile_pool(name="rb", bufs=2, side="right") as rb_pool, \
             tc.tile_pool(name="recip", bufs=2, side="right") as recip_pool, \
             tc.tile_pool(name="s_ps", bufs=2, space="PSUM", side="right") as s_ps, \
             tc.tile_pool(name="sum_ps", bufs=1, space="PSUM") as sum_ps, \
             tc.tile_pool(name="av_ps", bufs=1, space="PSUM") as av_ps:
            pend = []   # (h, pt, rb) awaiting attnV; emitted one head behind

            def emit_scores(h):
                pt = pt_pool.tile([128, NG, L], bf16, name=f"pt{h}", tag="pt")
                hs = slice(128 * h, 128 * (h + 1))
                ps_sum = sum_ps.tile([1, L], f32, tag="ps_sum")
                qrow = qT[:, h, :, :].rearrange("p g l -> p (g l)")

                def emit_sums(gk):
                    for j in range(2):
                        nc.tensor.matmul(ps_sum[:, 512 * j:512 * (j + 1)], ones_bf[:],
                                         pt[:, gk, 512 * j:512 * (j + 1)],
                                         start=(gk == 0), stop=(gk == NG - 1))

                for gk in range(NG):
                    ps_s = s_ps.tile([128, L], f32, tag="ps_s")
                    for j in range(2):
                        nc.tensor.matmul(ps_s[:, 512 * j:512 * (j + 1)],
                                         kT[:, gk, hs],
                                         qrow[:, 512 * j:512 * (j + 1)],
                                         start=True, stop=True)
                    nc.scalar.activation(out=pt[:, gk, :], in_=ps_s[:], func=AF.Exp,
                                         bias=0.0, scale=S2)
                    if gk > 0:
                        emit_sums(gk - 1)
                emit_sums(NG - 1)
                recip = recip_pool.tile([1, L], f32, tag="recip")
                nc.vector.reciprocal_approx_fast(out=recip[:], in_=ps_sum[:])
                rb = rb_pool.tile([128, L], f32, tag="rb")
                nc.gpsimd.partition_broadcast(rb[:], recip[:])
                pend.append((h, pt, rb))

            def emit_attnv():
                h, pt, rb = pend.pop(0)
                hs = slice(128 * h, 128 * (h + 1))
                ps_av = av_ps.tile([128, L], f32, tag="ps_av")
                for gk in range(NG):
                    for j in range(2):
                        nc.tensor.matmul(ps_av[:, 512 * j:512 * (j + 1)],
                                         v_bf[:, h, 128 * gk:128 * (gk + 1)],
                                         pt[:, gk, 512 * j:512 * (j + 1)],
                                         start=(gk == 0), stop=(gk == NG - 1))
                # attnT[:, g_q, 128h + l_r] = ps_av[:, (g_q, l_r)] * rb
                nc.vector.tensor_tensor(
                    out=attnT[:, :, hs],
                    in0=ps_av[:].rearrange("p (g l) -> p g l", g=NG),
                    in1=rb[:].rearrange("p (g l) -> p g l", g=NG), op=ALU.mult)

            for h in range(H):
                emit_scores(h)
                if pend and h > 0:
                    emit_attnv()
            while pend:
                emit_attnv()

        kT_pool.release()
        qT_pool.release()
        v_pool.release()

        # ---------------- Phase 6: output projection + residual ----------------
        with tc.tile_pool(name="otile", bufs=4, side="right") as ot_pool, \
             tc.tile_pool(name="out_ps", bufs=2, space="PSUM") as out_ps:
            bout_bc = ot_pool.tile([128, C], f32)
            nc.gpsimd.dma_start(out=bout_bc[:], in_=_bcast_ap(bout_d))
            for m in range(NT):
                ps_o = out_ps.tile([128, C], f32)
                for ki in range(NG):
                    lhsT = attnT[:, ki, 128 * m:128 * (m + 1)]
                    for j in range(2):
                        nc.tensor.matmul(
                            ps_o[:, 512 * j:512 * (j + 1)], lhsT,
                            wout_sb[:, ki, 512 * j:512 * (j + 1)],
                            start=(ki == 0), stop=(ki == NG - 1))
                t1 = ot_pool.tile([128, C], f32)
                nc.vector.tensor_tensor(out=t1[:], in0=ps_o[:], in1=xn[:, m, :],
                                        op=ALU.add)
                t2 = ot_pool.tile([128, C], f32)
                nc.vector.tensor_tensor(out=t2[:], in0=t1[:], in1=bout_bc[:],
                                        op=ALU.add)
                nc.sync.dma_start(out=out_d[128 * m:128 * (m + 1), :], in_=t2[:])

        wout_pool.release()

    return nc


_CACHE = {}


def _build(apply_affine: bool):
    key = apply_affine
    if key not in _CACHE:
        nc = bacc.Bacc("TRN2", target_bir_lowering=False, debug=False)
        _emit(nc, apply_affine)
        nc.compile()
        _CACHE[key] = nc
    return _CACHE[key]


def kernel(**inputs) -> np.ndarray:
    x = np.asarray(inputs["x"], np.float32)
    ln_g = np.asarray(inputs["ln_g"], np.float32)
    ln_b = np.asarray(inputs["ln_b"], np.float32)
    w_qkv = np.ascontiguousarray(np.asarray(inputs["w_qkv"], np.float32))
    b_qkv = np.asarray(inputs["b_qkv"], np.float32)
    w_out = np.ascontiguousarray(np.asarray(inputs["w_out"], np.float32))
    b_out = np.asarray(inputs["b_out"], np.float32)

    B = x.shape[0]
    assert x.shape == (B, L, C)
    apply_affine = not (np.all(ln_g == 1.0) and np.all(ln_b == 0.0))
    nc = _build(apply_affine)

    b_out_eff = b_out
    bqk_pre = np.ascontiguousarray(b_qkv[:2 * C].reshape(16, 128).T)
    bv = np.ascontiguousarray(b_qkv[2 * C:])
    w_out_bf = w_out.astype(ml_dtypes.bfloat16)

    in_maps = []
    for c in range(B):
        m = {
            "x": np.ascontiguousarray(x[c]),
            "w_qkv": w_qkv,
            "b_qk": bqk_pre,
            "b_v": bv,
            "w_out": w_out_bf,
            "b_out_eff": b_out_eff,
        }
        if apply_affine:
            m["ln_g"] = ln_g
            m["ln_b"] = ln_b
        in_maps.append(m)

    res = bass_utils.run_bass_kernel_spmd(nc, in_maps, core_ids=list(range(B)))
    return np.stack([res.results[c]["out"] for c in range(B)]).astype(np.float32)


# revision 42
# speedup vs baseline: 1.2671x; 1.0222x over previous
"""Trainium2 Bass kernel for an AttentionBlock (LN -> QKV -> attn -> out-proj + residual).

Shapes (hardcoded per problem spec): B=8, L=1024, C=1024, H=8 heads.
The reference uses a raw row-major reshape (torch-style .view) of q/k/v from
[B, L, C] to [B*H, L, C/H]; with L=1024, C=1024, H=8 this makes each
"attention head" operate on a contiguous 128-sequence-row block of the
[L, C] matrix, reinterpreted as [1024, 128].

Sharding: pure data-parallel over batch, one batch element per NeuronCore
(8 cores). No collectives.
"""

import math
from contextlib import ExitStack

import ml_dtypes
import numpy as np

import concourse.bass as bass
import concourse.bacc as bacc
import concourse.tile as tile
from concourse import mybir
from concourse import bass_utils
from concourse.masks import make_identity

L = 1024
C = 1024
H = 8          # heads; also number of 128-row l-tiles (head h <-> l-tile h)
CH = 128       # head dim
NT = 8         # l tiles (128 rows each)
NG = 8         # c groups (128 cols each)
EPS = 1e-5
S2 = 1.0 / math.sqrt(CH)   # combined q&k scale: (ch^-0.25)^2

f32 = mybir.dt.float32
f32r = mybir.dt.float32r
bf16 = mybir.dt.bfloat16
AF = mybir.ActivationFunctionType
ALU = mybir.AluOpType



def _bcast_ap(ap, p=128):
    """Broadcast a 1-D DRAM vector across p partitions (step-0 partition dim)."""
    return bass.AP(tensor=ap.tensor, offset=ap.offset, ap=[[0, p]] + list(ap.ap))


def _emit(nc, apply_affine: bool):
    x_d = nc.dram_tensor("x", [L, C], f32, kind="ExternalInput").ap()
    wqkv_d = nc.dram_tensor("w_qkv", [C, 3 * C], f32r, kind="ExternalInput").ap()
    bqk_d = nc.dram_tensor("b_qk", [128, 16], f32, kind="ExternalInput").ap()
    wout_d = nc.dram_tensor("w_out", [C, C], bf16, kind="ExternalInput").ap()
    bout_d = nc.dram_tensor("b_out_eff", [C], f32, kind="ExternalInput").ap()
    if apply_affine:
        g_d = nc.dram_tensor("ln_g", [C], f32, kind="ExternalInput").ap()
        b_d = nc.dram_tensor("ln_b", [C], f32, kind="ExternalInput").ap()
    out_d = nc.dram_tensor("out", [L, C], f32, kind="ExternalOutput").ap()

    with nc.allow_low_precision(reason="bf16/f32r compute by design"), \
         tile.TileContext(nc) as tc, ExitStack() as ctx:
        # Long-lived pools on the LEFT side, allocation order chosen so that
        # the ones dying earliest are on top of the stack.
        const = ctx.enter_context(tc.tile_pool(name="const", bufs=1, side="left"))
        ident = const.tile([128, 128], f32)
        make_identity(nc, ident)
        ones_bf = const.tile([128, 1], bf16)
        nc.vector.memset(ones_bf, 1.0)
        eps_sb = const.tile([128, 1], f32)
        nc.vector.memset(eps_sb, EPS)
        bqk_sb = const.tile([128, 16], f32)
        nc.sync.dma_start(out=bqk_sb[:], in_=bqk_d)
        bv_d = nc.dram_tensor("b_v", [C], f32, kind="ExternalInput").ap()
        bv_bc = const.tile([128, C], f32)
        nc.gpsimd.dma_start(out=bv_bc[:], in_=_bcast_ap(bv_d))
        if apply_affine:
            g_bc = const.tile([128, C], f32)
            nc.gpsimd.dma_start(out=g_bc[:], in_=_bcast_ap(g_d))
            b_bc = const.tile([128, C], f32)
            nc.gpsimd.dma_start(out=b_bc[:], in_=_bcast_ap(b_d))

        xn_pool = ctx.enter_context(tc.tile_pool(name="xn", bufs=1, side="left"))
        xn = xn_pool.tile([128, NT, C], f32)     # normalized x, natural [l, c]
        attnT_pool = ctx.enter_context(tc.tile_pool(name="attnT", bufs=1, side="left"))
        attnT = attnT_pool.tile([128, NG, L], bf16)   # [c', g_q, l]
        v_pool = tc.alloc_tile_pool(name="v", bufs=1, side="left")
        v_bf = v_pool.tile([128, NT, C], bf16)   # [l_r, l-tile, c]
        wv_pool = tc.alloc_tile_pool(name="wv", bufs=1, side="left")
        wv_sb = wv_pool.tile([128, NG, C], f32r)

        # ---------------- Phase 1: LayerNorm ----------------
        with tc.tile_pool(name="xin", bufs=8, side="right") as xin, \
             tc.tile_pool(name="lnst", bufs=4, side="right") as lnst, \
             tc.tile_pool(name="lntmp", bufs=3, side="right") as lntmp:
            for t in range(NT):
                xt = xin.tile([128, C], f32)
                stats = lnst.tile([128, 2, 6], f32)
                for j in range(2):
                    nc.sync.dma_start(
                        out=xt[:, 512 * j:512 * (j + 1)],
                        in_=x_d[128 * t:128 * (t + 1), 512 * j:512 * (j + 1)])
                    nc.vector.bn_stats(out=stats[:, j, :],
                                       in_=xt[:, 512 * j:512 * (j + 1)])
                mv = lnst.tile([128, 2], f32)
                nc.vector.bn_aggr(out=mv[:], in_=stats[:])
                sq = lnst.tile([128, 1], f32)
                nc.scalar.activation(out=sq[:], in_=mv[:, 1:2], func=AF.Sqrt,
                                     bias=eps_sb[:], scale=1.0)
                rstd = lnst.tile([128, 1], f32)
                nc.vector.reciprocal(out=rstd[:], in_=sq[:])
                nmr = lnst.tile([128, 1], f32)
                nc.vector.tensor_scalar(nmr[:], mv[:, 0:1], rstd[:], -1.0,
                                        ALU.mult, ALU.mult)
                if apply_affine:
                    zt = lntmp.tile([128, C], f32)
                    nc.scalar.activation(out=zt[:], in_=xt[:], func=AF.Identity,
                                         bias=nmr[:], scale=rstd[:])
                    zg = lntmp.tile([128, C], f32)
                    nc.vector.tensor_tensor(out=zg[:], in0=zt[:], in1=g_bc[:],
                                            op=ALU.mult)
                    nc.vector.tensor_tensor(out=xn[:, t, :], in0=zg[:], in1=b_bc[:],
                                            op=ALU.add)
                else:
                    nc.scalar.activation(out=xn[:, t, :], in_=xt[:], func=AF.Identity,
                                         bias=nmr[:], scale=rstd[:])

        # ------- Phase 2-4: transpose xn -> xnT; V, Q, K projections -------
        nc.sync.dma_start(
            out=wv_sb[:],
            in_=wqkv_d[:, 2 * C:3 * C].rearrange("(k p) n -> p k n", p=128))
        with tc.tile_pool(name="xnT", bufs=1, side="right") as xnT_pool:
            xnT = xnT_pool.tile([128, NG, L], f32r)   # [c', g, l]
            with tc.tile_pool(name="tr_ps", bufs=4, space="PSUM") as tr_ps:
                for t in range(NT):
                    for g in range(NG):
                        ps = tr_ps.tile([128, 128], f32)
                        nc.tensor.transpose(ps[:], xn[:, t, 128 * g:128 * (g + 1)],
                                            ident[:])
                        nc.scalar.copy(out=xnT[:, g, 128 * t:128 * (t + 1)], in_=ps[:])

            with tc.tile_pool(name="proj_ps", bufs=3, space="PSUM") as proj_ps:
                # V projection (natural layout, bf16 out)
                for m in range(NT):
                    psv = proj_ps.tile([128, C], f32, tag="proj")
                    for ki in range(NG):
                        lhsT = xnT[:, ki, 128 * m:128 * (m + 1)]
                        for j in range(2):
                            nc.tensor.matmul(
                                psv[:, 512 * j:512 * (j + 1)], lhsT,
                                wv_sb[:, ki, 512 * j:512 * (j + 1)],
                                start=(ki == 0), stop=(ki == NG - 1))
                    nc.vector.tensor_tensor(out=v_bf[:, m, :], in0=psv[:],
                                            in1=bv_bc[:], op=ALU.add)
                wv_pool.release()

                # Q, K projections (transposed layout)
                qT_pool = tc.alloc_tile_pool(name="qT", bufs=1, side="left")
                qT = qT_pool.tile([128, H, NG, 128], bf16)   # [c', h, g_q, l_r]
                kT_pool = tc.alloc_tile_pool(name="kT", bufs=1, side="left")
                kT = kT_pool.tile([128, NG, L], bf16)        # [c', g_k, l]
                with tc.tile_pool(name="wqk", bufs=6, side="right") as wqk_pool:
                    for co in range(16):
                        wslab = wqk_pool.tile([128, NG, 128], f32r)
                        nc.sync.dma_start(
                            out=wslab[:],
                            in_=wqkv_d[:, 128 * co:128 * (co + 1)].rearrange(
                                "(k p) n -> p k n", p=128))
                        psq = proj_ps.tile([128, L], f32, tag="proj")
                        for ki in range(NG):
                            for j in range(2):
                                nc.tensor.matmul(
                                    psq[:, 512 * j:512 * (j + 1)],
                                    wslab[:, ki, :],
                                    xnT[:, ki, 512 * j:512 * (j + 1)],
                                    start=(ki == 0), stop=(ki == NG - 1))
                        bias_col = bqk_sb[:, co:co + 1]
                        if co < 8:
                            # q: dst [c', h, l_r] over h (l = 128h + l_r)
                            nc.vector.tensor_scalar(
                                qT[:, :, co, :],
                                psq[:].rearrange("p (h l) -> p h l", h=H),
                                bias_col, None, ALU.add)
                        else:
                            nc.vector.tensor_scalar(kT[:, co - 8, :], psq[:],
                                                    bias_col, None, ALU.add)

        # ---------------- Phase 5: attention ----------------
        pt_bufs = 1 if apply_affine else 2
        wout_pool = tc.alloc_tile_pool(name="wout", bufs=1, side="right")
        wout_sb = wout_pool.tile([128, NG, C], bf16)
        nc.sync.dma_start(out=wout_sb[:],
                          in_=wout_d.rearrange("(k p) n -> p k n", p=128))
        with tc.tile_pool(name="pt", bufs=pt_bufs, side="right") as pt_pool, \
             tc.tile_pool(name="rb", bufs=2, side="right") as rb_pool, \
             tc.tile_pool(name="recip", bufs=2, side="right") as recip_pool, \
             tc.tile_pool(name="s_ps", bufs=2, space="PSUM", side="right") as s_ps, \
             tc.tile_pool(name="sum_ps", bufs=1, space="PSUM") as sum_ps, \
             tc.tile_pool(name="av_ps", bufs=1, space="PSUM") as av_ps:
            pend = []   # (h, pt, rb) awaiting attnV; emitted one head behind

            def emit_scores(h):
                pt = pt_pool.tile([128, NG, L], bf16, name=f"pt{h}", tag="pt")
                hs = slice(128 * h, 128 * (h + 1))
                ps_sum = sum_ps.tile([1, L], f32, tag="ps_sum")
                qrow = qT[:, h, :, :].rearrange("p g l -> p (g l)")

                def emit_sums(gk):
                    for j in range(2):
                        nc.tensor.matmul(ps_sum[:, 512 * j:512 * (j + 1)], ones_bf[:],
                                         pt[:, gk, 512 * j:512 * (j + 1)],
                                         start=(gk == 0), stop=(gk == NG - 1))

                for gk in range(NG):
                    ps_s = s_ps.tile([128, L], f32, tag="ps_s")
                    for j in range(2):
                        nc.tensor.matmul(ps_s[:, 512 * j:512 * (j + 1)],
                                         kT[:, gk, hs],
                                         qrow[:, 512 * j:512 * (j + 1)],
                                         start=True, stop=True)
                    nc.scalar.activation(out=pt[:, gk, :], in_=ps_s[:], func=AF.Exp,
                                         bias=0.0, scale=S2)
                    if gk > 0:
                        emit_sums(gk - 1)
                emit_sums(NG - 1)
                recip = recip_pool.tile([1, L], f32, tag="recip")
                nc.vector.reciprocal_approx_fast(out=recip[:], in_=ps_sum[:])
                rb = rb_pool.tile([128, L], f32, tag="rb")
                nc.gpsimd.partition_broadcast(rb[:], recip[:])
                pend.append((h, pt, rb))

            def emit_attnv():
                h, pt, rb = pend.pop(0)
                hs = slice(128 * h, 128 * (h + 1))
                ps_av = av_ps.tile([128, L], f32, tag="ps_av")
                for gk in range(NG):
                    for j in range(2):
                        nc.tensor.matmul(ps_av[:, 512 * j:512 * (j + 1)],
                                         v_bf[:, h, 128 * gk:128 * (gk + 1)],
                                         pt[:, gk, 512 * j:512 * (j + 1)],
                                         start=(gk == 0), stop=(gk == NG - 1))
                # attnT[:, g_q, 128h + l_r] = ps_av[:, (g_q, l_r)] * rb
                nc.vector.tensor_tensor(
                    out=attnT[:, :, hs],
                    in0=ps_av[:].rearrange("p (g l) -> p g l", g=NG),
                    in1=rb[:].rearrange("p (g l) -> p g l", g=NG), op=ALU.mult)

            for h in range(H):
                emit_scores(h)
                if pend and h > 0:
                    emit_attnv()
            while pend:
                emit_attnv()

        kT_pool.release()
        qT_pool.release()
        v_pool.release()

        # ---------------- Phase 6: output projection + residual ----------------
        with tc.tile_pool(name="otile", bufs=4, side="right") as ot_pool, \
             tc.tile_pool(name="out_ps", bufs=2, space="PSUM") as out_ps:
            bout_bc = ot_pool.tile([128, C], f32)
            nc.gpsimd.dma_start(out=bout_bc[:], in_=_bcast_ap(bout_d))
            for m in range(NT):
                ps_o = out_ps.tile([128, C], f32)
                for ki in range(NG):
                    lhsT = attnT[:, ki, 128 * m:128 * (m + 1)]
                    for j in range(2):
                        nc.tensor.matmul(
                            ps_o[:, 512 * j:512 * (j + 1)], lhsT,
                            wout_sb[:, ki, 512 * j:512 * (j + 1)],
                            start=(ki == 0), stop=(ki == NG - 1))
                t1 = ot_pool.tile([128, C], f32)
                nc.vector.tensor_tensor(out=t1[:], in0=ps_o[:], in1=xn[:, m, :],
                                        op=ALU.add)
                t2 = ot_pool.tile([128, C], f32)
                nc.vector.tensor_tensor(out=t2[:], in0=t1[:], in1=bout_bc[:],
                                        op=ALU.add)
                nc.sync.dma_start(out=out_d[128 * m:128 * (m + 1), :], in_=t2[:])

        wout_pool.release()

    return nc


_CACHE = {}


def _build(apply_affine: bool):
    key = apply_affine
    if key not in _CACHE:
        nc = bacc.Bacc("TRN2", target_bir_lowering=False, debug=False)
        _emit(nc, apply_affine)
        nc.compile()
        _CACHE[key] = nc
    return _CACHE[key]


def kernel(**inputs) -> np.ndarray:
    x = np.asarray(inputs["x"], np.float32)
    ln_g = np.asarray(inputs["ln_g"], np.float32)
    ln_b = np.asarray(inputs["ln_b"], np.float32)
    w_qkv = np.ascontiguousarray(np.asarray(inputs["w_qkv"], np.float32))
    b_qkv = np.asarray(inputs["b_qkv"], np.float32)
    w_out = np.ascontiguousarray(np.asarray(inputs["w_out"], np.float32))
    b_out = np.asarray(inputs["b_out"], np.float32)

    B = x.shape[0]
    assert x.shape == (B, L, C)
    apply_affine = not (np.all(ln_g == 1.0) and np.all(ln_b == 0.0))
    nc = _build(apply_affine)

    b_out_eff = b_out
    bqk_pre = np.ascontiguousarray(b_qkv[:2 * C].reshape(16, 128).T)
    bv = np.ascontiguousarray(b_qkv[2 * C:])
    w_out_bf = w_out.astype(ml_dtypes.bfloat16)

    in_maps = []
    for c in range(B):
        m = {
            "x": np.ascontiguousarray(x[c]),
            "w_qkv": w_qkv,
            "b_qk": bqk_pre,
            "b_v": bv,
            "w_out": w_out_bf,
            "b_out_eff": b_out_eff,
        }
        if apply_affine:
            m["ln_g"] = ln_g
            m["ln_b"] = ln_b
        in_maps.append(m)

    res = bass_utils.run_bass_kernel_spmd(nc, in_maps, core_ids=list(range(B)))
    return np.stack([res.results[c]["out"] for c in range(B)]).astype(np.float32)


# revision 43
# speedup vs baseline: 1.2728x; 1.0045x over previous
"""Trainium2 Bass kernel for an AttentionBlock (LN -> QKV -> attn -> out-proj + residual).

Shapes (hardcoded per problem spec): B=8, L=1024, C=1024, H=8 heads.
The reference uses a raw row-major reshape (torch-style .view) of q/k/v from
[B, L, C] to [B*H, L, C/H]; with L=1024, C=1024, H=8 this makes each
"attention head" operate on a contiguous 128-sequence-row block of the
[L, C] matrix, reinterpreted as [1024, 128].

Sharding: pure data-parallel over batch, one batch element per NeuronCore
(8 cores). No collectives.
"""

import math
from contextlib import ExitStack

import ml_dtypes
import numpy as np

import concourse.bass as bass
import concourse.bacc as bacc
import concourse.tile as tile
from concourse import mybir
from concourse import bass_utils
from concourse.masks import make_identity

L = 1024
C = 1024
H = 8          # heads; also number of 128-row l-tiles (head h <-> l-tile h)
CH = 128       # head dim
NT = 8         # l tiles (128 rows each)
NG = 8         # c groups (128 cols each)
EPS = 1e-5
S2 = 1.0 / math.sqrt(CH)   # combined q&k scale: (ch^-0.25)^2

f32 = mybir.dt.float32
f32r = mybir.dt.float32r
bf16 = mybir.dt.bfloat16
AF = mybir.ActivationFunctionType
ALU = mybir.AluOpType



def _bcast_ap(ap, p=128):
    """Broadcast a 1-D DRAM vector across p partitions (step-0 partition dim)."""
    return bass.AP(tensor=ap.tensor, offset=ap.offset, ap=[[0, p]] + list(ap.ap))


def _emit(nc, apply_affine: bool):
    x_d = nc.dram_tensor("x", [L, C], f32, kind="ExternalInput").ap()
    wqkv_d = nc.dram_tensor("w_qkv", [C, 3 * C], f32r, kind="ExternalInput").ap()
    bqk_d = nc.dram_tensor("b_qk", [128, 16], f32, kind="ExternalInput").ap()
    wout_d = nc.dram_tensor("w_out", [C, C], bf16, kind="ExternalInput").ap()
    bout_d = nc.dram_tensor("b_out_eff", [C], f32, kind="ExternalInput").ap()
    if apply_affine:
        g_d = nc.dram_tensor("ln_g", [C], f32, kind="ExternalInput").ap()
        b_d = nc.dram_tensor("ln_b", [C], f32, kind="ExternalInput").ap()
    out_d = nc.dram_tensor("out", [L, C], f32, kind="ExternalOutput").ap()

    with nc.allow_low_precision(reason="bf16/f32r compute by design"), \
         tile.TileContext(nc) as tc, ExitStack() as ctx:
        # Long-lived pools on the LEFT side, allocation order chosen so that
        # the ones dying earliest are on top of the stack.
        const = ctx.enter_context(tc.tile_pool(name="const", bufs=1, side="left"))
        ident = const.tile([128, 128], f32)
        make_identity(nc, ident)
        ones_bf = const.tile([128, 1], bf16)
        nc.vector.memset(ones_bf, 1.0)
        eps_sb = const.tile([128, 1], f32)
        nc.vector.memset(eps_sb, EPS)
        bqk_sb = const.tile([128, 16], f32)
        nc.sync.dma_start(out=bqk_sb[:], in_=bqk_d)
        bv_d = nc.dram_tensor("b_v", [C], f32, kind="ExternalInput").ap()
        bv_bc = const.tile([128, C], f32)
        nc.gpsimd.dma_start(out=bv_bc[:], in_=_bcast_ap(bv_d))
        if apply_affine:
            g_bc = const.tile([128, C], f32)
            nc.gpsimd.dma_start(out=g_bc[:], in_=_bcast_ap(g_d))
            b_bc = const.tile([128, C], f32)
            nc.gpsimd.dma_start(out=b_bc[:], in_=_bcast_ap(b_d))

        xn_pool = ctx.enter_context(tc.tile_pool(name="xn", bufs=1, side="left"))
        xn = xn_pool.tile([128, NT, C], f32)     # normalized x, natural [l, c]
        attnT_pool = ctx.enter_context(tc.tile_pool(name="attnT", bufs=1, side="left"))
        attnT = attnT_pool.tile([128, NG, L], bf16)   # [c', g_q, l]
        v_pool = tc.alloc_tile_pool(name="v", bufs=1, side="left")
        v_bf = v_pool.tile([128, NT, C], bf16)   # [l_r, l-tile, c]
        wv_pool = tc.alloc_tile_pool(name="wv", bufs=1, side="left")
        wv_sb = wv_pool.tile([128, NG, C], f32r)

        # ---------------- Phase 1: LayerNorm ----------------
        with tc.tile_pool(name="xin", bufs=8, side="right") as xin, \
             tc.tile_pool(name="lnst", bufs=4, side="right") as lnst, \
             tc.tile_pool(name="lntmp", bufs=3, side="right") as lntmp:
            for t in range(NT):
                xt = xin.tile([128, C], f32)
                stats = lnst.tile([128, 2, 6], f32)
                for j in range(2):
                    nc.sync.dma_start(
                        out=xt[:, 512 * j:512 * (j + 1)],
                        in_=x_d[128 * t:128 * (t + 1), 512 * j:512 * (j + 1)])
                    nc.vector.bn_stats(out=stats[:, j, :],
                                       in_=xt[:, 512 * j:512 * (j + 1)])
                mv = lnst.tile([128, 2], f32)
                nc.vector.bn_aggr(out=mv[:], in_=stats[:])
                sq = lnst.tile([128, 1], f32)
                nc.scalar.activation(out=sq[:], in_=mv[:, 1:2], func=AF.Sqrt,
                                     bias=eps_sb[:], scale=1.0)
                rstd = lnst.tile([128, 1], f32)
                nc.vector.reciprocal(out=rstd[:], in_=sq[:])
                nmr = lnst.tile([128, 1], f32)
                nc.vector.tensor_scalar(nmr[:], mv[:, 0:1], rstd[:], -1.0,
                                        ALU.mult, ALU.mult)
                if apply_affine:
                    zt = lntmp.tile([128, C], f32)
                    nc.scalar.activation(out=zt[:], in_=xt[:], func=AF.Identity,
                                         bias=nmr[:], scale=rstd[:])
                    zg = lntmp.tile([128, C], f32)
                    nc.vector.tensor_tensor(out=zg[:], in0=zt[:], in1=g_bc[:],
                                            op=ALU.mult)
                    nc.vector.tensor_tensor(out=xn[:, t, :], in0=zg[:], in1=b_bc[:],
                                            op=ALU.add)
                else:
                    nc.scalar.activation(out=xn[:, t, :], in_=xt[:], func=AF.Identity,
                                         bias=nmr[:], scale=rstd[:])

        # ------- Phase 2-4: transpose xn -> xnT; V, Q, K projections -------
        nc.sync.dma_start(
            out=wv_sb[:],
            in_=wqkv_d[:, 2 * C:3 * C].rearrange("(k p) n -> p k n", p=128))
        with tc.tile_pool(name="xnT", bufs=1, side="right") as xnT_pool:
            xnT = xnT_pool.tile([128, NG, L], f32r)   # [c', g, l]
            with tc.tile_pool(name="tr_ps", bufs=4, space="PSUM") as tr_ps:
                for t in range(NT):
                    for g in range(NG):
                        ps = tr_ps.tile([128, 128], f32)
                        nc.tensor.transpose(ps[:], xn[:, t, 128 * g:128 * (g + 1)],
                                            ident[:])
                        nc.scalar.copy(out=xnT[:, g, 128 * t:128 * (t + 1)], in_=ps[:])

            with tc.tile_pool(name="proj_ps", bufs=4, space="PSUM") as proj_ps:
                # V projection (natural layout, bf16 out)
                for m in range(NT):
                    psv = proj_ps.tile([128, C], f32, tag="proj")
                    for ki in range(NG):
                        lhsT = xnT[:, ki, 128 * m:128 * (m + 1)]
                        for j in range(2):
                            nc.tensor.matmul(
                                psv[:, 512 * j:512 * (j + 1)], lhsT,
                                wv_sb[:, ki, 512 * j:512 * (j + 1)],
                                start=(ki == 0), stop=(ki == NG - 1))
                    nc.vector.tensor_tensor(out=v_bf[:, m, :], in0=psv[:],
                                            in1=bv_bc[:], op=ALU.add)
                wv_pool.release()

                # Q, K projections (transposed layout)
                qT_pool = tc.alloc_tile_pool(name="qT", bufs=1, side="left")
                qT = qT_pool.tile([128, H, NG, 128], bf16)   # [c', h, g_q, l_r]
                kT_pool = tc.alloc_tile_pool(name="kT", bufs=1, side="left")
                kT = kT_pool.tile([128, NG, L], bf16)        # [c', g_k, l]
                with tc.tile_pool(name="wqk", bufs=8, side="right") as wqk_pool:
                    for co in range(16):
                        wslab = wqk_pool.tile([128, NG, 128], f32r)
                        nc.sync.dma_start(
                            out=wslab[:],
                            in_=wqkv_d[:, 128 * co:128 * (co + 1)].rearrange(
                                "(k p) n -> p k n", p=128))
                        psq = proj_ps.tile([128, L], f32, tag="proj")
                        for ki in range(NG):
                            for j in range(2):
                                nc.tensor.matmul(
                                    psq[:, 512 * j:512 * (j + 1)],
                                    wslab[:, ki, :],
                                    xnT[:, ki, 512 * j:512 * (j + 1)],
                                    start=(ki == 0), stop=(ki == NG - 1))
                        bias_col = bqk_sb[:, co:co + 1]
                        if co < 8:
                            # q: dst [c', h, l_r] over h (l = 128h + l_r)
                            nc.vector.tensor_scalar(
                                qT[:, :, co, :],
                                psq[:].rearrange("p (h l) -> p h l", h=H),
                                bias_col, None, ALU.add)
                        else:
                            nc.vector.tensor_scalar(kT[:, co - 8, :], psq[:],
                                                    bias_col, None, ALU.add)

        # ---------------- Phase 5: attention ----------------
        pt_bufs = 1 if apply_affine else 2
        wout_pool = tc.alloc_tile_pool(name="wout", bufs=1, side="right")
        wout_sb = wout_pool.tile([128, NG, C], bf16)
        nc.sync.dma_start(out=wout_sb[:],
                          in_=wout_d.rearrange("(k p) n -> p k n", p=128))
        with tc.tile_pool(name="pt", bufs=pt_bufs, side="right") as pt_pool, \
             tc.tile_pool(name="rb", bufs=3, side="right") as rb_pool, \
             tc.tile_pool(name="recip", bufs=3, side="right") as recip_pool, \
             tc.tile_pool(name="s_ps", bufs=2, space="PSUM", side="right") as s_ps, \
             tc.tile_pool(name="sum_ps", bufs=1, space="PSUM") as sum_ps, \
             tc.tile_pool(name="av_ps", bufs=1, space="PSUM") as av_ps:
            pend = []   # (h, pt, rb) awaiting attnV; emitted one head behind

            def emit_scores(h):
                pt = pt_pool.tile([128, NG, L], bf16, name=f"pt{h}", tag="pt")
                hs = slice(128 * h, 128 * (h + 1))
                ps_sum = sum_ps.tile([1, L], f32, tag="ps_sum")
                qrow = qT[:, h, :, :].rearrange("p g l -> p (g l)")

                def emit_sums(gk):
                    for j in range(2):
                        nc.tensor.matmul(ps_sum[:, 512 * j:512 * (j + 1)], ones_bf[:],
                                         pt[:, gk, 512 * j:512 * (j + 1)],
                                         start=(gk == 0), stop=(gk == NG - 1))

                for gk in range(NG):
                    ps_s = s_ps.tile([128, L], f32, tag="ps_s")
                    for j in range(2):
                        nc.tensor.matmul(ps_s[:, 512 * j:512 * (j + 1)],
                                         kT[:, gk, hs],
                                         qrow[:, 512 * j:512 * (j + 1)],
                                         start=True, stop=True)
                    nc.scalar.activation(out=pt[:, gk, :], in_=ps_s[:], func=AF.Exp,
                                         bias=0.0, scale=S2)
                    if gk > 0:
                        emit_sums(gk - 1)
                emit_sums(NG - 1)
                recip = recip_pool.tile([1, L], f32, tag="recip")
                nc.vector.reciprocal_approx_fast(out=recip[:], in_=ps_sum[:])
                rb = rb_pool.tile([128, L], f32, tag="rb")
                nc.gpsimd.partition_broadcast(rb[:], recip[:])
                pend.append((h, pt, rb))

            def emit_attnv():
                h, pt, rb = pend.pop(0)
                hs = slice(128 * h, 128 * (h + 1))
                ps_av = av_ps.tile([128, L], f32, tag="ps_av")
                for gk in range(NG):
                    for j in range(2):
                        nc.tensor.matmul(ps_av[:, 512 * j:512 * (j + 1)],
                                         v_bf[:, h, 128 * gk:128 * (gk + 1)],
                                         pt[:, gk, 512 * j:512 * (j + 1)],
                                         start=(gk == 0), stop=(gk == NG - 1))
                # attnT[:, g_q, 128h + l_r] = ps_av[:, (g_q, l_r)] * rb
                nc.vector.tensor_tensor(
                    out=attnT[:, :, hs],
                    in0=ps_av[:].rearrange("p (g l) -> p g l", g=NG),
                    in1=rb[:].rearrange("p (g l) -> p g l", g=NG), op=ALU.mult)

            for h in range(H):
                emit_scores(h)
                if pend and h > 0:
                    emit_attnv()
            while pend:
                emit_attnv()

        kT_pool.release()
        qT_pool.release()
        v_pool.release()

        # ---------------- Phase 6: output projection + residual ----------------
        with tc.tile_pool(name="otile", bufs=4, side="right") as ot_pool, \
             tc.tile_pool(name="out_ps", bufs=2, space="PSUM") as out_ps:
            bout_bc = ot_pool.tile([128, C], f32)
            nc.gpsimd.dma_start(out=bout_bc[:], in_=_bcast_ap(bout_d))
            for m in range(NT):
                ps_o = out_ps.tile([128, C], f32)
                for ki in range(NG):
                    lhsT = attnT[:, ki, 128 * m:128 * (m + 1)]
                    for j in range(2):
                        nc.tensor.matmul(
                            ps_o[:, 512 * j:512 * (j + 1)], lhsT,
                            wout_sb[:, ki, 512 * j:512 * (j + 1)],
                            start=(ki == 0), stop=(ki == NG - 1))
                t1 = ot_pool.tile([128, C], f32)
                nc.vector.tensor_tensor(out=t1[:], in0=ps_o[:], in1=xn[:, m, :],
                                        op=ALU.add)
                t2 = ot_pool.tile([128, C], f32)
                nc.vector.tensor_tensor(out=t2[:], in0=t1[:], in1=bout_bc[:],
                                        op=ALU.add)
                nc.sync.dma_start(out=out_d[128 * m:128 * (m + 1), :], in_=t2[:])

        wout_pool.release()

    return nc


_CACHE = {}


def _build(apply_affine: bool):
    key = apply_affine
    if key not in _CACHE:
        nc = bacc.Bacc("TRN2", target_bir_lowering=False, debug=False)
        _emit(nc, apply_affine)
        nc.compile()
        _CACHE[key] = nc
    return _CACHE[key]


def kernel(**inputs) -> np.ndarray:
    x = np.asarray(inputs["x"], np.float32)
    ln_g = np.asarray(inputs["ln_g"], np.float32)
    ln_b = np.asarray(inputs["ln_b"], np.float32)
    w_qkv = np.ascontiguousarray(np.asarray(inputs["w_qkv"], np.float32))
    b_qkv = np.asarray(inputs["b_qkv"], np.float32)
    w_out = np.ascontiguousarray(np.asarray(inputs["w_out"], np.float32))
    b_out = np.asarray(inputs["b_out"], np.float32)

    B = x.shape[0]
    assert x.shape == (B, L, C)
    apply_affine = not (np.all(ln_g == 1.0) and np.all(ln_b == 0.0))
    nc = _build(apply_affine)

    b_out_eff = b_out
    bqk_pre = np.ascontiguousarray(b_qkv[:2 * C].reshape(16, 128).T)
    bv = np.ascontiguousarray(b_qkv[2 * C:])
    w_out_bf = w_out.astype(ml_dtypes.bfloat16)

    in_maps = []
    for c in range(B):
        m = {
            "x": np.ascontiguousarray(x[c]),
            "w_qkv": w_qkv,
            "b_qk": bqk_pre,
            "b_v": bv,
            "w_out": w_out_bf,
            "b_out_eff": b_out_eff,
        }
        if apply_affine:
            m["ln_g"] = ln_g
            m["ln_b"] = ln_b
        in_maps.append(m)

    res = bass_utils.run_bass_kernel_spmd(nc, in_maps, core_ids=list(range(B)))
    return np.stack([res.results[c]["out"] for c in range(B)]).astype(np.float32)


# revision 44
# speedup vs baseline: 1.2813x; 1.0067x over previous
"""Trainium2 Bass kernel for an AttentionBlock (LN -> QKV -> attn -> out-proj + residual).

Shapes (hardcoded per problem spec): B=8, L=1024, C=1024, H=8 heads.
The reference uses a raw row-major reshape (torch-style .view) of q/k/v from
[B, L, C] to [B*H, L, C/H]; with L=1024, C=1024, H=8 this makes each
"attention head" operate on a contiguous 128-sequence-row block of the
[L, C] matrix, reinterpreted as [1024, 128].

Sharding: pure data-parallel over batch, one batch element per NeuronCore
(8 cores). No collectives.
"""

import math
from contextlib import ExitStack

import ml_dtypes
import numpy as np

import concourse.bass as bass
import concourse.bacc as bacc
import concourse.tile as tile
from concourse import mybir
from concourse import bass_utils
from concourse.masks import make_identity

L = 1024
C = 1024
H = 8          # heads; also number of 128-row l-tiles (head h <-> l-tile h)
CH = 128       # head dim
NT = 8         # l tiles (128 rows each)
NG = 8         # c groups (128 cols each)
EPS = 1e-5
S2 = 1.0 / math.sqrt(CH)   # combined q&k scale: (ch^-0.25)^2

f32 = mybir.dt.float32
f32r = mybir.dt.float32r
bf16 = mybir.dt.bfloat16
AF = mybir.ActivationFunctionType
ALU = mybir.AluOpType



def _bcast_ap(ap, p=128):
    """Broadcast a 1-D DRAM vector across p partitions (step-0 partition dim)."""
    return bass.AP(tensor=ap.tensor, offset=ap.offset, ap=[[0, p]] + list(ap.ap))


def _emit(nc, apply_affine: bool):
    x_d = nc.dram_tensor("x", [L, C], f32, kind="ExternalInput").ap()
    wqkv_d = nc.dram_tensor("w_qkv", [C, 3 * C], f32r, kind="ExternalInput").ap()
    bqk_d = nc.dram_tensor("b_qk", [128, 16], f32, kind="ExternalInput").ap()
    wout_d = nc.dram_tensor("w_out", [C, C], bf16, kind="ExternalInput").ap()
    bout_d = nc.dram_tensor("b_out_eff", [C], f32, kind="ExternalInput").ap()
    if apply_affine:
        g_d = nc.dram_tensor("ln_g", [C], f32, kind="ExternalInput").ap()
        b_d = nc.dram_tensor("ln_b", [C], f32, kind="ExternalInput").ap()
    out_d = nc.dram_tensor("out", [L, C], f32, kind="ExternalOutput").ap()

    with nc.allow_low_precision(reason="bf16/f32r compute by design"), \
         tile.TileContext(nc) as tc, ExitStack() as ctx:
        # Long-lived pools on the LEFT side, allocation order chosen so that
        # the ones dying earliest are on top of the stack.
        const = ctx.enter_context(tc.tile_pool(name="const", bufs=1, side="left"))
        ident = const.tile([128, 128], f32)
        make_identity(nc, ident)
        ones_bf = const.tile([128, 1], bf16)
        nc.vector.memset(ones_bf, 1.0)
        eps_sb = const.tile([128, 1], f32)
        nc.vector.memset(eps_sb, EPS)
        bqk_sb = const.tile([128, 16], f32)
        nc.sync.dma_start(out=bqk_sb[:], in_=bqk_d)
        bv_d = nc.dram_tensor("b_v", [C], f32, kind="ExternalInput").ap()
        bv_bc = const.tile([128, C], f32)
        nc.gpsimd.dma_start(out=bv_bc[:], in_=_bcast_ap(bv_d))
        if apply_affine:
            g_bc = const.tile([128, C], f32)
            nc.gpsimd.dma_start(out=g_bc[:], in_=_bcast_ap(g_d))
            b_bc = const.tile([128, C], f32)
            nc.gpsimd.dma_start(out=b_bc[:], in_=_bcast_ap(b_d))

        xn_pool = ctx.enter_context(tc.tile_pool(name="xn", bufs=1, side="left"))
        xn = xn_pool.tile([128, NT, C], f32)     # normalized x, natural [l, c]
        attnT_pool = ctx.enter_context(tc.tile_pool(name="attnT", bufs=1, side="left"))
        attnT = attnT_pool.tile([128, NG, L], bf16)   # [c', g_q, l]
        v_pool = tc.alloc_tile_pool(name="v", bufs=1, side="left")
        v_bf = v_pool.tile([128, NT, C], bf16)   # [l_r, l-tile, c]
        wv_pool = tc.alloc_tile_pool(name="wv", bufs=1, side="left")
        wv_sb = wv_pool.tile([128, NG, C], f32r)

        # ---------------- Phase 1: LayerNorm ----------------
        with tc.tile_pool(name="xin", bufs=8, side="right") as xin, \
             tc.tile_pool(name="lnst", bufs=4, side="right") as lnst, \
             tc.tile_pool(name="lntmp", bufs=3, side="right") as lntmp:
            for t in range(NT):
                xt = xin.tile([128, C], f32)
                stats = lnst.tile([128, 2, 6], f32)
                for j in range(2):
                    nc.sync.dma_start(
                        out=xt[:, 512 * j:512 * (j + 1)],
                        in_=x_d[128 * t:128 * (t + 1), 512 * j:512 * (j + 1)])
                    nc.vector.bn_stats(out=stats[:, j, :],
                                       in_=xt[:, 512 * j:512 * (j + 1)])
                mv = lnst.tile([128, 2], f32)
                nc.vector.bn_aggr(out=mv[:], in_=stats[:])
                sq = lnst.tile([128, 1], f32)
                nc.scalar.activation(out=sq[:], in_=mv[:, 1:2], func=AF.Sqrt,
                                     bias=eps_sb[:], scale=1.0)
                rstd = lnst.tile([128, 1], f32)
                nc.vector.reciprocal(out=rstd[:], in_=sq[:])
                nmr = lnst.tile([128, 1], f32)
                nc.vector.tensor_scalar(nmr[:], mv[:, 0:1], rstd[:], -1.0,
                                        ALU.mult, ALU.mult)
                if apply_affine:
                    zt = lntmp.tile([128, C], f32)
                    nc.scalar.activation(out=zt[:], in_=xt[:], func=AF.Identity,
                                         bias=nmr[:], scale=rstd[:])
                    zg = lntmp.tile([128, C], f32)
                    nc.vector.tensor_tensor(out=zg[:], in0=zt[:], in1=g_bc[:],
                                            op=ALU.mult)
                    nc.vector.tensor_tensor(out=xn[:, t, :], in0=zg[:], in1=b_bc[:],
                                            op=ALU.add)
                else:
                    nc.scalar.activation(out=xn[:, t, :], in_=xt[:], func=AF.Identity,
                                         bias=nmr[:], scale=rstd[:])

        # ------- Phase 2-4: transpose xn -> xnT; V, Q, K projections -------
        nc.sync.dma_start(
            out=wv_sb[:],
            in_=wqkv_d[:, 2 * C:3 * C].rearrange("(k p) n -> p k n", p=128))
        with tc.tile_pool(name="xnT", bufs=1, side="right") as xnT_pool:
            xnT = xnT_pool.tile([128, NG, L], f32r)   # [c', g, l]
            with tc.tile_pool(name="tr_ps", bufs=4, space="PSUM") as tr_ps:
                for t in range(NT):
                    for g in range(NG):
                        ps = tr_ps.tile([128, 128], f32)
                        nc.tensor.transpose(ps[:], xn[:, t, 128 * g:128 * (g + 1)],
                                            ident[:])
                        nc.scalar.copy(out=xnT[:, g, 128 * t:128 * (t + 1)], in_=ps[:])

            with tc.tile_pool(name="proj_ps", bufs=4, space="PSUM") as proj_ps:
                # V projection (natural layout, bf16 out)
                for m in range(NT):
                    psv = proj_ps.tile([128, C], f32, tag="proj")
                    for ki in range(NG):
                        lhsT = xnT[:, ki, 128 * m:128 * (m + 1)]
                        for j in range(2):
                            nc.tensor.matmul(
                                psv[:, 512 * j:512 * (j + 1)], lhsT,
                                wv_sb[:, ki, 512 * j:512 * (j + 1)],
                                start=(ki == 0), stop=(ki == NG - 1))
                    nc.vector.tensor_tensor(out=v_bf[:, m, :], in0=psv[:],
                                            in1=bv_bc[:], op=ALU.add)
                wv_pool.release()

                # Q, K projections (transposed layout)
                qT_pool = tc.alloc_tile_pool(name="qT", bufs=1, side="left")
                qT = qT_pool.tile([128, H, NG, 128], bf16)   # [c', h, g_q, l_r]
                kT_pool = tc.alloc_tile_pool(name="kT", bufs=1, side="left")
                kT = kT_pool.tile([128, NG, L], bf16)        # [c', g_k, l]
                with tc.tile_pool(name="wqk", bufs=8, side="right") as wqk_pool:
                    for co in range(16):
                        wslab = wqk_pool.tile([128, NG, 128], f32r)
                        nc.sync.dma_start(
                            out=wslab[:],
                            in_=wqkv_d[:, 128 * co:128 * (co + 1)].rearrange(
                                "(k p) n -> p k n", p=128))
                        psq = proj_ps.tile([128, L], f32, tag="proj")
                        for ki in range(NG):
                            for j in range(2):
                                nc.tensor.matmul(
                                    psq[:, 512 * j:512 * (j + 1)],
                                    wslab[:, ki, :],
                                    xnT[:, ki, 512 * j:512 * (j + 1)],
                                    start=(ki == 0), stop=(ki == NG - 1))
                        bias_col = bqk_sb[:, co:co + 1]
                        if co < 8:
                            # q: dst [c', h, l_r] over h (l = 128h + l_r)
                            nc.vector.tensor_scalar(
                                qT[:, :, co, :],
                                psq[:].rearrange("p (h l) -> p h l", h=H),
                                bias_col, None, ALU.add)
                        else:
                            nc.vector.tensor_scalar(kT[:, co - 8, :], psq[:],
                                                    bias_col, None, ALU.add)

        # ---------------- Phase 5: attention ----------------
        pt_bufs = 2 if apply_affine else 3
        wout_pool = tc.alloc_tile_pool(name="wout", bufs=1, side="right")
        wout_sb = wout_pool.tile([128, NG, C], bf16)
        nc.sync.dma_start(out=wout_sb[:],
                          in_=wout_d.rearrange("(k p) n -> p k n", p=128))
        with tc.tile_pool(name="pt", bufs=pt_bufs, side="right") as pt_pool, \
             tc.tile_pool(name="rb", bufs=3, side="right") as rb_pool, \
             tc.tile_pool(name="recip", bufs=3, side="right") as recip_pool, \
             tc.tile_pool(name="s_ps", bufs=2, space="PSUM", side="right") as s_ps, \
             tc.tile_pool(name="sum_ps", bufs=1, space="PSUM") as sum_ps, \
             tc.tile_pool(name="av_ps", bufs=1, space="PSUM") as av_ps:
            pend = []   # (h, pt, rb) awaiting attnV; emitted one head behind

            def emit_scores(h):
                pt = pt_pool.tile([128, NG, L], bf16, name=f"pt{h}", tag="pt")
                hs = slice(128 * h, 128 * (h + 1))
                ps_sum = sum_ps.tile([1, L], f32, tag="ps_sum")
                qrow = qT[:, h, :, :].rearrange("p g l -> p (g l)")

                def emit_sums(gk):
                    for j in range(2):
                        nc.tensor.matmul(ps_sum[:, 512 * j:512 * (j + 1)], ones_bf[:],
                                         pt[:, gk, 512 * j:512 * (j + 1)],
                                         start=(gk == 0), stop=(gk == NG - 1))

                for gk in range(NG):
                    ps_s = s_ps.tile([128, L], f32, tag="ps_s")
                    for j in range(2):
                        nc.tensor.matmul(ps_s[:, 512 * j:512 * (j + 1)],
                                         kT[:, gk, hs],
                                         qrow[:, 512 * j:512 * (j + 1)],
                                         start=True, stop=True)
                    nc.scalar.activation(out=pt[:, gk, :], in_=ps_s[:], func=AF.Exp,
                                         bias=0.0, scale=S2)
                    if gk > 0:
                        emit_sums(gk - 1)
                emit_sums(NG - 1)
                recip = recip_pool.tile([1, L], f32, tag="recip")
                nc.vector.reciprocal_approx_fast(out=recip[:], in_=ps_sum[:])
                rb = rb_pool.tile([128, L], f32, tag="rb")
                nc.gpsimd.partition_broadcast(rb[:], recip[:])
                pend.append((h, pt, rb))

            def emit_attnv():
                h, pt, rb = pend.pop(0)
                hs = slice(128 * h, 128 * (h + 1))
                ps_av = av_ps.tile([128, L], f32, tag="ps_av")
                for gk in range(NG):
                    for j in range(2):
                        nc.tensor.matmul(ps_av[:, 512 * j:512 * (j + 1)],
                                         v_bf[:, h, 128 * gk:128 * (gk + 1)],
                                         pt[:, gk, 512 * j:512 * (j + 1)],
                                         start=(gk == 0), stop=(gk == NG - 1))
                # attnT[:, g_q, 128h + l_r] = ps_av[:, (g_q, l_r)] * rb
                nc.vector.tensor_tensor(
                    out=attnT[:, :, hs],
                    in0=ps_av[:].rearrange("p (g l) -> p g l", g=NG),
                    in1=rb[:].rearrange("p (g l) -> p g l", g=NG), op=ALU.mult)

            for h in range(H):
                emit_scores(h)
                if pend and h > 0:
                    emit_attnv()
            while pend:
                emit_attnv()

        kT_pool.release()
        qT_pool.release()
        v_pool.release()

        # ---------------- Phase 6: output projection + residual ----------------
        with tc.tile_pool(name="otile", bufs=4, side="right") as ot_pool, \
             tc.tile_pool(name="out_ps", bufs=2, space="PSUM") as out_ps:
            bout_bc = ot_pool.tile([128, C], f32)
            nc.gpsimd.dma_start(out=bout_bc[:], in_=_bcast_ap(bout_d))
            for m in range(NT):
                ps_o = out_ps.tile([128, C], f32)
                for ki in range(NG):
                    lhsT = attnT[:, ki, 128 * m:128 * (m + 1)]
                    for j in range(2):
                        nc.tensor.matmul(
                            ps_o[:, 512 * j:512 * (j + 1)], lhsT,
                            wout_sb[:, ki, 512 * j:512 * (j + 1)],
                            start=(ki == 0), stop=(ki == NG - 1))
                t1 = ot_pool.tile([128, C], f32)
                nc.vector.tensor_tensor(out=t1[:], in0=ps_o[:], in1=xn[:, m, :],
                                        op=ALU.add)
                t2 = ot_pool.tile([128, C], f32)
                nc.vector.tensor_tensor(out=t2[:], in0=t1[:], in1=bout_bc[:],
                                        op=ALU.add)
                nc.sync.dma_start(out=out_d[128 * m:128 * (m + 1), :], in_=t2[:])

        wout_pool.release()

    return nc


_CACHE = {}


def _build(apply_affine: bool):
    key = apply_affine
    if key not in _CACHE:
        nc = bacc.Bacc("TRN2", target_bir_lowering=False, debug=False)
        _emit(nc, apply_affine)
        nc.compile()
        _CACHE[key] = nc
    return _CACHE[key]


def kernel(**inputs) -> np.ndarray:
    x = np.asarray(inputs["x"], np.float32)
    ln_g = np.asarray(inputs["ln_g"], np.float32)
    ln_b = np.asarray(inputs["ln_b"], np.float32)
    w_qkv = np.ascontiguousarray(np.asarray(inputs["w_qkv"], np.float32))
    b_qkv = np.asarray(inputs["b_qkv"], np.float32)
    w_out = np.ascontiguousarray(np.asarray(inputs["w_out"], np.float32))
    b_out = np.asarray(inputs["b_out"], np.float32)

    B = x.shape[0]
    assert x.shape == (B, L, C)
    apply_affine = not (np.all(ln_g == 1.0) and np.all(ln_b == 0.0))
    nc = _build(apply_affine)

    b_out_eff = b_out
    bqk_pre = np.ascontiguousarray(b_qkv[:2 * C].reshape(16, 128).T)
    bv = np.ascontiguousarray(b_qkv[2 * C:])
    w_out_bf = w_out.astype(ml_dtypes.bfloat16)

    in_maps = []
    for c in range(B):
        m = {
            "x": np.ascontiguousarray(x[c]),
            "w_qkv": w_qkv,
            "b_qk": bqk_pre,
            "b_v": bv,
            "w_out": w_out_bf,
            "b_out_eff": b_out_eff,
        }
        if apply_affine:
            m["ln_g"] = ln_g
            m["ln_b"] = ln_b
        in_maps.append(m)

    res = bass_utils.run_bass_kernel_spmd(nc, in_maps, core_ids=list(range(B)))
    return np.stack([res.results[c]["out"] for c in range(B)]).astype(np.float32)


# revision 45
# speedup vs baseline: 1.3026x; 1.0166x over previous
"""Trainium2 Bass kernel for an AttentionBlock (LN -> QKV -> attn -> out-proj + residual).

Shapes (hardcoded per problem spec): B=8, L=1024, C=1024, H=8 heads.
The reference uses a raw row-major reshape (torch-style .view) of q/k/v from
[B, L, C] to [B*H, L, C/H]; with L=1024, C=1024, H=8 this makes each
"attention head" operate on a contiguous 128-sequence-row block of the
[L, C] matrix, reinterpreted as [1024, 128].

Sharding: pure data-parallel over batch, one batch element per NeuronCore
(8 cores). No collectives.
"""

import math
from contextlib import ExitStack

import ml_dtypes
import numpy as np

import concourse.bass as bass
import concourse.bacc as bacc
import concourse.tile as tile
from concourse import mybir
from concourse import bass_utils
from concourse.masks import make_identity

L = 1024
C = 1024
H = 8          # heads; also number of 128-row l-tiles (head h <-> l-tile h)
CH = 128       # head dim
NT = 8         # l tiles (128 rows each)
NG = 8         # c groups (128 cols each)
EPS = 1e-5
S2 = 1.0 / math.sqrt(CH)   # combined q&k scale: (ch^-0.25)^2

f32 = mybir.dt.float32
f32r = mybir.dt.float32r
bf16 = mybir.dt.bfloat16
AF = mybir.ActivationFunctionType
ALU = mybir.AluOpType



def _bcast_ap(ap, p=128):
    """Broadcast a 1-D DRAM vector across p partitions (step-0 partition dim)."""
    return bass.AP(tensor=ap.tensor, offset=ap.offset, ap=[[0, p]] + list(ap.ap))


def _emit(nc, apply_affine: bool):
    x_d = nc.dram_tensor("x", [L, C], f32, kind="ExternalInput").ap()
    wqkv_d = nc.dram_tensor("w_qkv", [C, 3 * C], f32r, kind="ExternalInput").ap()
    bqk_d = nc.dram_tensor("b_qk", [128, 16], f32, kind="ExternalInput").ap()
    wout_d = nc.dram_tensor("w_out", [C, C], bf16, kind="ExternalInput").ap()
    bout_d = nc.dram_tensor("b_out_eff", [C], f32, kind="ExternalInput").ap()
    if apply_affine:
        g_d = nc.dram_tensor("ln_g", [C], f32, kind="ExternalInput").ap()
        b_d = nc.dram_tensor("ln_b", [C], f32, kind="ExternalInput").ap()
    out_d = nc.dram_tensor("out", [L, C], f32, kind="ExternalOutput").ap()

    with nc.allow_low_precision(reason="bf16/f32r compute by design"), \
         tile.TileContext(nc) as tc, ExitStack() as ctx:
        # Long-lived pools on the LEFT side, allocation order chosen so that
        # the ones dying earliest are on top of the stack.
        const = ctx.enter_context(tc.tile_pool(name="const", bufs=1, side="left"))
        ident = const.tile([128, 128], f32)
        make_identity(nc, ident)
        ones_bf = const.tile([128, 1], bf16)
        nc.vector.memset(ones_bf, 1.0)
        eps_sb = const.tile([128, 1], f32)
        nc.vector.memset(eps_sb, EPS)
        bqk_sb = const.tile([128, 16], f32)
        nc.sync.dma_start(out=bqk_sb[:], in_=bqk_d)
        bv_d = nc.dram_tensor("b_v", [C], f32, kind="ExternalInput").ap()
        bv_bc = const.tile([128, C], f32)
        nc.gpsimd.dma_start(out=bv_bc[:], in_=_bcast_ap(bv_d))
        if apply_affine:
            g_bc = const.tile([128, C], f32)
            nc.gpsimd.dma_start(out=g_bc[:], in_=_bcast_ap(g_d))
            b_bc = const.tile([128, C], f32)
            nc.gpsimd.dma_start(out=b_bc[:], in_=_bcast_ap(b_d))

        xn_pool = ctx.enter_context(tc.tile_pool(name="xn", bufs=1, side="left"))
        xn = xn_pool.tile([128, NT, C], f32)     # normalized x, natural [l, c]
        attnT_pool = ctx.enter_context(tc.tile_pool(name="attnT", bufs=1, side="left"))
        attnT = attnT_pool.tile([128, NG, L], bf16)   # [c', g_q, l]
        v_pool = tc.alloc_tile_pool(name="v", bufs=1, side="left")
        v_bf = v_pool.tile([128, NT, C], bf16)   # [l_r, l-tile, c]
        wv_pool = tc.alloc_tile_pool(name="wv", bufs=1, side="left")
        wv_sb = wv_pool.tile([128, NG, C], f32r)

        # ---------------- Phase 1: LayerNorm ----------------
        with tc.tile_pool(name="xin", bufs=8, side="right") as xin, \
             tc.tile_pool(name="lnst", bufs=4, side="right") as lnst, \
             tc.tile_pool(name="lntmp", bufs=3, side="right") as lntmp:
            for t in range(NT):
                xt = xin.tile([128, C], f32)
                stats = lnst.tile([128, 2, 6], f32)
                for j in range(2):
                    nc.sync.dma_start(
                        out=xt[:, 512 * j:512 * (j + 1)],
                        in_=x_d[128 * t:128 * (t + 1), 512 * j:512 * (j + 1)])
                    nc.vector.bn_stats(out=stats[:, j, :],
                                       in_=xt[:, 512 * j:512 * (j + 1)])
                mv = lnst.tile([128, 2], f32)
                nc.vector.bn_aggr(out=mv[:], in_=stats[:])
                sq = lnst.tile([128, 1], f32)
                nc.scalar.activation(out=sq[:], in_=mv[:, 1:2], func=AF.Sqrt,
                                     bias=eps_sb[:], scale=1.0)
                rstd = lnst.tile([128, 1], f32)
                nc.vector.reciprocal(out=rstd[:], in_=sq[:])
                nmr = lnst.tile([128, 1], f32)
                nc.vector.tensor_scalar(nmr[:], mv[:, 0:1], rstd[:], -1.0,
                                        ALU.mult, ALU.mult)
                if apply_affine:
                    zt = lntmp.tile([128, C], f32)
                    nc.scalar.activation(out=zt[:], in_=xt[:], func=AF.Identity,
                                         bias=nmr[:], scale=rstd[:])
                    zg = lntmp.tile([128, C], f32)
                    nc.vector.tensor_tensor(out=zg[:], in0=zt[:], in1=g_bc[:],
                                            op=ALU.mult)
                    nc.vector.tensor_tensor(out=xn[:, t, :], in0=zg[:], in1=b_bc[:],
                                            op=ALU.add)
                else:
                    nc.scalar.activation(out=xn[:, t, :], in_=xt[:], func=AF.Identity,
                                         bias=nmr[:], scale=rstd[:])

        # ------- Phase 2-4: transpose xn -> xnT; V, Q, K projections -------
        nc.sync.dma_start(
            out=wv_sb[:],
            in_=wqkv_d[:, 2 * C:3 * C].rearrange("(k p) n -> p k n", p=128))
        with tc.tile_pool(name="xnT", bufs=1, side="right") as xnT_pool:
            xnT = xnT_pool.tile([128, NG, L], f32r)   # [c', g, l]
            with tc.tile_pool(name="tr_ps", bufs=6, space="PSUM") as tr_ps:
                for t in range(NT):
                    for g in range(NG):
                        ps = tr_ps.tile([128, 128], f32)
                        nc.tensor.transpose(ps[:], xn[:, t, 128 * g:128 * (g + 1)],
                                            ident[:])
                        dst = xnT[:, g, 128 * t:128 * (t + 1)]
                        if g % 2 == 0:
                            nc.scalar.copy(out=dst, in_=ps[:])
                        else:
                            nc.vector.tensor_copy(dst, ps[:])

            with tc.tile_pool(name="proj_ps", bufs=4, space="PSUM") as proj_ps:
                # V projection (natural layout, bf16 out)
                for m in range(NT):
                    psv = proj_ps.tile([128, C], f32, tag="proj")
                    for ki in range(NG):
                        lhsT = xnT[:, ki, 128 * m:128 * (m + 1)]
                        for j in range(2):
                            nc.tensor.matmul(
                                psv[:, 512 * j:512 * (j + 1)], lhsT,
                                wv_sb[:, ki, 512 * j:512 * (j + 1)],
                                start=(ki == 0), stop=(ki == NG - 1))
                    nc.vector.tensor_tensor(out=v_bf[:, m, :], in0=psv[:],
                                            in1=bv_bc[:], op=ALU.add)
                wv_pool.release()

                # Q, K projections (transposed layout)
                qT_pool = tc.alloc_tile_pool(name="qT", bufs=1, side="left")
                qT = qT_pool.tile([128, H, NG, 128], bf16)   # [c', h, g_q, l_r]
                kT_pool = tc.alloc_tile_pool(name="kT", bufs=1, side="left")
                kT = kT_pool.tile([128, NG, L], bf16)        # [c', g_k, l]
                with tc.tile_pool(name="wqk", bufs=8, side="right") as wqk_pool:
                    for co in range(16):
                        wslab = wqk_pool.tile([128, NG, 128], f32r)
                        nc.sync.dma_start(
                            out=wslab[:],
                            in_=wqkv_d[:, 128 * co:128 * (co + 1)].rearrange(
                                "(k p) n -> p k n", p=128))
                        psq = proj_ps.tile([128, L], f32, tag="proj")
                        for ki in range(NG):
                            for j in range(2):
                                nc.tensor.matmul(
                                    psq[:, 512 * j:512 * (j + 1)],
                                    wslab[:, ki, :],
                                    xnT[:, ki, 512 * j:512 * (j + 1)],
                                    start=(ki == 0), stop=(ki == NG - 1))
                        bias_col = bqk_sb[:, co:co + 1]
                        if co < 8:
                            # q: dst [c', h, l_r] over h (l = 128h + l_r)
                            nc.vector.tensor_scalar(
                                qT[:, :, co, :],
                                psq[:].rearrange("p (h l) -> p h l", h=H),
                                bias_col, None, ALU.add)
                        else:
                            nc.vector.tensor_scalar(kT[:, co - 8, :], psq[:],
                                                    bias_col, None, ALU.add)

        # ---------------- Phase 5: attention ----------------
        pt_bufs = 2 if apply_affine else 3
        wout_pool = tc.alloc_tile_pool(name="wout", bufs=1, side="right")
        wout_sb = wout_pool.tile([128, NG, C], bf16)
        nc.sync.dma_start(out=wout_sb[:],
                          in_=wout_d.rearrange("(k p) n -> p k n", p=128))
        with tc.tile_pool(name="pt", bufs=pt_bufs, side="right") as pt_pool, \
             tc.tile_pool(name="rb", bufs=3, side="right") as rb_pool, \
             tc.tile_pool(name="recip", bufs=3, side="right") as recip_pool, \
             tc.tile_pool(name="s_ps", bufs=2, space="PSUM", side="right") as s_ps, \
             tc.tile_pool(name="sum_ps", bufs=1, space="PSUM") as sum_ps, \
             tc.tile_pool(name="av_ps", bufs=1, space="PSUM") as av_ps:
            pend = []   # (h, pt, rb) awaiting attnV; emitted one head behind

            def emit_scores(h):
                pt = pt_pool.tile([128, NG, L], bf16, name=f"pt{h}", tag="pt")
                hs = slice(128 * h, 128 * (h + 1))
                ps_sum = sum_ps.tile([1, L], f32, tag="ps_sum")
                qrow = qT[:, h, :, :].rearrange("p g l -> p (g l)")

                def emit_sums(gk):
                    for j in range(2):
                        nc.tensor.matmul(ps_sum[:, 512 * j:512 * (j + 1)], ones_bf[:],
                                         pt[:, gk, 512 * j:512 * (j + 1)],
                                         start=(gk == 0), stop=(gk == NG - 1))

                for gk in range(NG):
                    ps_s = s_ps.tile([128, L], f32, tag="ps_s")
                    for j in range(2):
                        nc.tensor.matmul(ps_s[:, 512 * j:512 * (j + 1)],
                                         kT[:, gk, hs],
                                         qrow[:, 512 * j:512 * (j + 1)],
                                         start=True, stop=True)
                    nc.scalar.activation(out=pt[:, gk, :], in_=ps_s[:], func=AF.Exp,
                                         bias=0.0, scale=S2)
                    if gk > 0:
                        emit_sums(gk - 1)
                emit_sums(NG - 1)
                recip = recip_pool.tile([1, L], f32, tag="recip")
                nc.vector.reciprocal_approx_fast(out=recip[:], in_=ps_sum[:])
                rb = rb_pool.tile([128, L], f32, tag="rb")
                nc.gpsimd.partition_broadcast(rb[:], recip[:])
                pend.append((h, pt, rb))

            def emit_attnv():
                h, pt, rb = pend.pop(0)
                hs = slice(128 * h, 128 * (h + 1))
                ps_av = av_ps.tile([128, L], f32, tag="ps_av")
                for gk in range(NG):
                    for j in range(2):
                        nc.tensor.matmul(ps_av[:, 512 * j:512 * (j + 1)],
                                         v_bf[:, h, 128 * gk:128 * (gk + 1)],
                                         pt[:, gk, 512 * j:512 * (j + 1)],
                                         start=(gk == 0), stop=(gk == NG - 1))
                # attnT[:, g_q, 128h + l_r] = ps_av[:, (g_q, l_r)] * rb
                nc.vector.tensor_tensor(
                    out=attnT[:, :, hs],
                    in0=ps_av[:].rearrange("p (g l) -> p g l", g=NG),
                    in1=rb[:].rearrange("p (g l) -> p g l", g=NG), op=ALU.mult)

            for h in range(H):
                emit_scores(h)
                if pend and h > 0:
                    emit_attnv()
            while pend:
                emit_attnv()

        kT_pool.release()
        qT_pool.release()
        v_pool.release()

        # ---------------- Phase 6: output projection + residual ----------------
        with tc.tile_pool(name="otile", bufs=4, side="right") as ot_pool, \
             tc.tile_pool(name="out_ps", bufs=3, space="PSUM") as out_ps:
            bout_bc = ot_pool.tile([128, C], f32)
            nc.gpsimd.dma_start(out=bout_bc[:], in_=_bcast_ap(bout_d))
            for m in range(NT):
                ps_o = out_ps.tile([128, C], f32)
                for ki in range(NG):
                    lhsT = attnT[:, ki, 128 * m:128 * (m + 1)]
                    for j in range(2):
                        nc.tensor.matmul(
                            ps_o[:, 512 * j:512 * (j + 1)], lhsT,
                            wout_sb[:, ki, 512 * j:512 * (j + 1)],
                            start=(ki == 0), stop=(ki == NG - 1))
                t1 = ot_pool.tile([128, C], f32)
                nc.vector.tensor_tensor(out=t1[:], in0=ps_o[:], in1=xn[:, m, :],
                                        op=ALU.add)
                t2 = ot_pool.tile([128, C], f32)
                nc.vector.tensor_tensor(out=t2[:], in0=t1[:], in1=bout_bc[:],
                                        op=ALU.add)
                nc.sync.dma_start(out=out_d[128 * m:128 * (m + 1), :], in_=t2[:])

        wout_pool.release()

    return nc


_CACHE = {}


def _build(apply_affine: bool):
    key = apply_affine
    if key not in _CACHE:
        nc = bacc.Bacc("TRN2", target_bir_lowering=False, debug=False)
        _emit(nc, apply_affine)
        nc.compile()
        _CACHE[key] = nc
    return _CACHE[key]


def kernel(**inputs) -> np.ndarray:
    x = np.asarray(inputs["x"], np.float32)
    ln_g = np.asarray(inputs["ln_g"], np.float32)
    ln_b = np.asarray(inputs["ln_b"], np.float32)
    w_qkv = np.ascontiguousarray(np.asarray(inputs["w_qkv"], np.float32))
    b_qkv = np.asarray(inputs["b_qkv"], np.float32)
    w_out = np.ascontiguousarray(np.asarray(inputs["w_out"], np.float32))
    b_out = np.asarray(inputs["b_out"], np.float32)

    B = x.shape[0]
    assert x.shape == (B, L, C)
    apply_affine = not (np.all(ln_g == 1.0) and np.all(ln_b == 0.0))
    nc = _build(apply_affine)

    b_out_eff = b_out
    bqk_pre = np.ascontiguousarray(b_qkv[:2 * C].reshape(16, 128).T)
    bv = np.ascontiguousarray(b_qkv[2 * C:])
    w_out_bf = w_out.astype(ml_dtypes.bfloat16)

    in_maps = []
    for c in range(B):
        m = {
            "x": np.ascontiguousarray(x[c]),
            "w_qkv": w_qkv,
            "b_qk": bqk_pre,
            "b_v": bv,
            "w_out": w_out_bf,
            "b_out_eff": b_out_eff,
        }
        if apply_affine:
            m["ln_g"] = ln_g
            m["ln_b"] = ln_b
        in_maps.append(m)

    res = bass_utils.run_bass_kernel_spmd(nc, in_maps, core_ids=list(range(B)))
    return np.stack([res.results[c]["out"] for c in range(B)]).astype(np.float32)
